# revision 16
# baseline (speedup 1.0000x reference)
"""Trainium2 Bass kernel for ADRiverDynamics (gnn_message_passing).

8 independent point clouds (B*L=8), one per NeuronCore (pure data parallel),
plus one tiny AllReduce for global BatchNorm statistics.

Per-core pipeline (cloud of N=3072 points, C=64 channels, K=16 neighbors):
  S0  load f/xyz, weights; build combined DRAM rows [f|xyz|pad] for gathers
  S1  PE transposes (fT, xyzT->A/B), head convs (flow/diff/unc), gate conv
  S2  pass A: negD = -dist^2 via matmul; per-row top-16 via max8/match_replace
  S3  neighbor gather via SWDGE dma_gather (512B rows)
  S4  pass C: K-dense math (cos/softmax weights), fused weighted aggregation
  S5  reaction conv + global-batch BN (AllReduce) + relu + conv
  S6  combine: out = f + dt*(adv + diff + reac)
"""
import functools
import numpy as np

B, L, N, C, K = 2, 4, 3072, 64, 16
NB = N // 128          # 24 point blocks
TAU = 0.15
BN_EPS = 1e-5
NCORES = 8
BT = 4                 # blocks per pass-C slice
NSL = NB // BT         # pass-C slices

WEIGHT_NAMES = ["Wf", "bf", "Wd", "bd", "Wu", "bu", "Wg1", "bg1", "Wg2", "bg2",
                "Wgate", "bgate", "Wr1", "br1", "gamma", "beta", "Wr2", "br2",
                "log_dt"]


def _build(debug=False):
    import contextlib
    from concourse import bacc
    import concourse.bass as bass
    import concourse.tile as tile
    import concourse.mybir as mybir
    from concourse import masks

    f32 = mybir.dt.float32
    u16 = mybir.dt.uint16
    i16 = mybir.dt.int16
    Alu = mybir.AluOpType
    Act = mybir.ActivationFunctionType
    AX = mybir.AxisListType
    AP = bass.AP

    nc = bacc.Bacc("TRN2", target_bir_lowering=False, debug=False,
                   num_devices=NCORES)

    f_ext = nc.dram_tensor("f", [N, C], f32, kind="ExternalInput")
    xyz_ext = nc.dram_tensor("xyz", [N, 3], f32, kind="ExternalInput")
    wshapes = {"Wf": [3, C], "bf": [3], "Wd": [1, C], "bd": [1], "Wu": [1, C],
               "bu": [1], "Wg1": [C, 3], "bg1": [C], "Wg2": [C, C], "bg2": [C],
               "Wgate": [C, C], "bgate": [C], "Wr1": [C, C + 5], "br1": [C],
               "gamma": [C], "beta": [C], "Wr2": [C, C], "br2": [C],
               "log_dt": [1]}
    w_ext = {k: nc.dram_tensor(k, shp, f32, kind="ExternalInput")
             for k, shp in wshapes.items()}
    out_ext = nc.dram_tensor("out", [N, C], f32, kind="ExternalOutput")
    dbg_ext = {}
    if debug:
        for k, shp in {"d_idx": [128, NB * K], "d_agg": [128, NB * C],
                       "d_de": [128, NB], "d_dist": [128, NB * 2],
                       "d_bn": [C, 4], "d_heads": [5, N],
                       "d_negd": [128, N], "d_num": [128, NB * K],
                       "d_uw": [128, NB * K], "d_fnei": [128, BT * K * 128],
                       "d_vhat": [128, NB * 3]}.items():
            dbg_ext[k] = nc.dram_tensor(k, shp, f32, kind="ExternalOutput")

    with tile.TileContext(nc) as tc:
        class _Stacks(contextlib.ExitStack):
            def __init__(self):
                super().__init__()
                self._pa = contextlib.ExitStack()
                self._pc = contextlib.ExitStack()
            def enter_pa(self, cm):
                return self._pa.enter_context(cm)
            def enter_pc(self, cm):
                return self._pc.enter_context(cm)
            def close_pa(self):
                self._pa.close()
            def close_pc(self):
                self._pc.close()
            def __exit__(self, *a):
                self._pc.close()
                self._pa.close()
                return super().__exit__(*a)
        ctx = _Stacks()
        with ctx:
            cpool = ctx.enter_context(tc.tile_pool(name="consts", bufs=1))
            big = ctx.enter_context(tc.tile_pool(name="big", bufs=1))
            dram = ctx.enter_context(tc.tile_pool(name="dram", bufs=1, space="DRAM"))
            psum = ctx.enter_context(tc.tile_pool(name="psum", bufs=4, space="PSUM"))
            small = ctx.enter_context(tc.tile_pool(name="small", bufs=1))

            def ps(p, fr):
                return psum.tile([p, fr], f32, tag="ps", name="pst")

            # ---------------- constants / weights -----------------
            ident = cpool.tile([128, 128], f32)
            masks.make_identity(nc, ident[:])

            WhT = cpool.tile([C, 5], f32)
            nc.sync.dma_start(WhT[:, 0:3], AP(w_ext["Wf"], 0, [[1, C], [C, 3]]))
            nc.sync.dma_start(WhT[:, 3:4], AP(w_ext["Wd"], 0, [[1, C], [C, 1]]))
            nc.sync.dma_start(WhT[:, 4:5], AP(w_ext["Wu"], 0, [[1, C], [C, 1]]))
            bhead = cpool.tile([5, 1], f32)
            nc.sync.dma_start(bhead[0:3, :], AP(w_ext["bf"], 0, [[1, 3], [1, 1]]))
            nc.sync.dma_start(bhead[3:4, :], AP(w_ext["bd"], 0, [[1, 1], [1, 1]]))
            nc.sync.dma_start(bhead[4:5, :], AP(w_ext["bu"], 0, [[1, 1], [1, 1]]))

            WgateT = cpool.tile([C, C], f32)
            nc.sync.dma_start(WgateT[:], AP(w_ext["Wgate"], 0, [[1, C], [C, C]]))
            Wg1T = cpool.tile([3, C], f32)
            nc.sync.dma_start(Wg1T[:], AP(w_ext["Wg1"], 0, [[1, 3], [3, C]]))
            Wg2T = cpool.tile([C, C], f32)
            nc.sync.dma_start(Wg2T[:], AP(w_ext["Wg2"], 0, [[1, C], [C, C]]))
            Wr1fT = cpool.tile([C, C], f32)
            nc.sync.dma_start(Wr1fT[:], AP(w_ext["Wr1"], 0, [[1, C], [C + 5, C]]))
            Wr1flT = cpool.tile([3, C], f32)
            nc.sync.dma_start(Wr1flT[:], AP(w_ext["Wr1"], C, [[1, 3], [C + 5, C]]))
            Wr1dT = cpool.tile([2, C], f32)
            nc.sync.dma_start(Wr1dT[:], AP(w_ext["Wr1"], C + 3, [[1, 2], [C + 5, C]]))
            Wr2T = cpool.tile([C, C], f32)
            nc.sync.dma_start(Wr2T[:], AP(w_ext["Wr2"], 0, [[1, C], [C, C]]))

            def vec_col(name):
                t = cpool.tile([C, 1], f32, tag=name, name=name + "_v")
                nc.sync.dma_start(t[:], AP(w_ext[name], 0, [[1, C], [1, 1]]))
                return t
            bgate_v = vec_col("bgate")
            bg1_v = vec_col("bg1")
            bg2_v = vec_col("bg2")
            br2_v = vec_col("br2")
            gamma_v = vec_col("gamma")
            beta_v = vec_col("beta")

            zero128 = cpool.tile([128, 1], f32)
            nc.vector.memset(zero128[:], 0.0)

            dtv = cpool.tile([128, 1], f32)
            nc.sync.dma_start(dtv[:], AP(w_ext["log_dt"], 0, [[0, 128], [1, 1]]))
            nc.scalar.activation(dtv[:], dtv[:], Act.Exp, bias=zero128[:], scale=1.0)
            nc.vector.tensor_scalar(dtv[:], dtv[:], 1e-4, 10.0, Alu.max, Alu.min)

            # ---------------- S0 loads -----------------
            f_sb = big.tile([128, NB, C], f32)
            nc.sync.dma_start(f_sb[:], AP(f_ext, 0, [[C, 128], [128 * C, NB], [1, C]]))
            xyz_sb = big.tile([128, NB, 3], f32)
            nc.sync.dma_start(xyz_sb[:], AP(xyz_ext, 0, [[3, 128], [128 * 3, NB], [1, 3]]))


            # ---------------- S1 transposes + convs -----------------
            fxT = big.tile([128, N], f32)
            fT = fxT[0:C, :]
            for j in range(6):
                pt = ps(C, 512)
                for q in range(4):
                    b = 4 * j + q
                    nc.tensor.matmul(pt[:, 128 * q:128 * (q + 1)],
                                     f_sb[:, b:b + 1, :], ident[:, :],
                                     is_transpose=True)
                nc.scalar.copy(fxT[0:C, 512 * j:512 * (j + 1)], pt[:])

            pa = ctx.enter_pa(tc.tile_pool(name="passa", bufs=2))
            A1_m = pa.tile([3, N], f32, tag="A1_m", bufs=1)   # 2x
            B1_m = pa.tile([3, N], f32, tag="B1_m", bufs=1)   # x
            A2_m = pa.tile([2, N], f32, tag="A2_m", bufs=1)   # [-sq; -1]
            B2_m = pa.tile([2, N], f32, tag="B2_m", bufs=1)   # [1; sq]
            nc.vector.memset(A2_m[:], -1.0)   # row 1 keeps -1
            nc.vector.memset(B2_m[:], 1.0)    # row 0 keeps +1
            for j in range(6):
                pt = ps(3, 512)
                for q in range(4):
                    b = 4 * j + q
                    nc.tensor.matmul(pt[:, 128 * q:128 * (q + 1)],
                                     xyz_sb[:, b:b + 1, :], ident[:, :],
                                     is_transpose=True)
                nc.scalar.mul(A1_m[:, 512 * j:512 * (j + 1)], pt[:], 2.0)
                nc.vector.tensor_copy(B1_m[:, 512 * j:512 * (j + 1)], pt[:])
                nc.scalar.copy(fxT[C:C + 3, 512 * j:512 * (j + 1)], pt[:])

            xyz2 = small.tile([128, NB, 3], f32)
            nc.vector.tensor_tensor(xyz2[:], xyz_sb[:], xyz_sb[:], Alu.mult)
            sq_p = small.tile([128, NB, 1], f32)
            nc.vector.tensor_reduce(sq_p[:], xyz2[:], axis=AX.X, op=Alu.add)
            sqn_p = small.tile([128, NB, 1], f32)
            nc.vector.tensor_scalar(sqn_p[:], sq_p[:], -1.0, None, Alu.mult)
            pt = ps(NB, 128)
            nc.tensor.matmul(pt[:], sq_p[:], ident[:, :], is_transpose=True)
            sq24 = small.tile([NB, 128], f32)
            nc.vector.tensor_copy(sq24[:], pt[:])
            pt = ps(NB, 128)
            nc.tensor.matmul(pt[:], sqn_p[:], ident[:, :], is_transpose=True)
            sqn24 = small.tile([NB, 128], f32)
            nc.vector.tensor_copy(sqn24[:], pt[:])
            nc.sync.dma_start(B2_m[1:2, :], sq24[:])
            nc.sync.dma_start(A2_m[0:1, :], sqn24[:])

            headsT = big.tile([5, N], f32)
            gateT = big.tile([C, N], f32)
            for j in range(6):
                sl = slice(512 * j, 512 * (j + 1))
                ph = ps(5, 512)
                nc.tensor.matmul(ph[:], WhT[:], fT[:, sl], start=True, stop=True)
                nc.vector.tensor_scalar(headsT[:, sl], ph[:], bhead[:], None, Alu.add)
                pg = ps(C, 512)
                nc.tensor.matmul(pg[:], WgateT[:], fT[:, sl], start=True, stop=True)
                nc.scalar.activation(gateT[:, sl], pg[:], Act.Sigmoid,
                                     bias=bgate_v[:], scale=1.0)

            hp = small.tile([128, NB, 5], f32)
            pt5 = ps(128, NB * 5)
            for b in range(NB):
                nc.tensor.matmul(pt5[:, 5 * b:5 * (b + 1)],
                                 headsT[:, 128 * b:128 * (b + 1)], ident[0:5, 0:5],
                                 is_transpose=True)
            nc.vector.tensor_copy(hp[:], pt5[:])

            flow_p = hp[:, :, 0:3]
            de = small.tile([128, NB, 1], f32)
            tmp_b = small.tile([128, NB, 1], f32)
            nc.scalar.activation(tmp_b[:], hp[:, :, 3:4], Act.Exp,
                                 bias=zero128[:], scale=1.0)
            nc.vector.tensor_scalar(tmp_b[:], tmp_b[:], 1.0, None, Alu.add)
            nc.scalar.activation(tmp_b[:], tmp_b[:], Act.Ln,
                                 bias=zero128[:], scale=1.0)
            sgu = small.tile([128, NB, 1], f32)
            nc.scalar.activation(sgu[:], hp[:, :, 4:5], Act.Sigmoid,
                                 bias=zero128[:], scale=1.0)
            nc.vector.tensor_scalar(sgu[:], sgu[:], 1.0, None, Alu.add)
            nc.vector.tensor_tensor(de[:], tmp_b[:], sgu[:], Alu.mult)
            de16 = small.tile([128, NB, 1], f32)
            nc.vector.tensor_scalar(de16[:], de[:], 1.0 / K, None, Alu.mult)

            fl2 = small.tile([128, NB, 3], f32)
            nc.vector.tensor_tensor(fl2[:], flow_p, flow_p, Alu.mult)
            vn = small.tile([128, NB, 1], f32)
            nc.vector.tensor_reduce(vn[:], fl2[:], axis=AX.X, op=Alu.add)
            nc.scalar.activation(vn[:], vn[:], Act.Sqrt, bias=zero128[:], scale=1.0)
            nc.vector.tensor_scalar(vn[:], vn[:], 1e-6, None, Alu.max)
            rv = small.tile([128, NB, 1], f32)
            nc.vector.reciprocal(rv[:], vn[:])
            vhat = small.tile([128, NB, 3], f32)
            nc.vector.tensor_tensor(vhat[:], flow_p,
                                    rv[:].broadcast_to((128, NB, 3)), Alu.mult)

            fgm = small.tile([3, 1], f32)
            nc.vector.tensor_reduce(fgm[:], headsT[0:3, :], axis=AX.X, op=Alu.add)
            nc.vector.tensor_scalar(fgm[:], fgm[:], 1.0 / N, None, Alu.mult)
            pg1 = ps(C, 1)
            nc.tensor.matmul(pg1[:], Wg1T[:], fgm[:], start=True, stop=True)
            hg = small.tile([C, 1], f32)
            nc.scalar.activation(hg[:], pg1[:], Act.Relu, bias=bg1_v[:], scale=1.0)
            pg2 = ps(C, 1)
            nc.tensor.matmul(pg2[:], Wg2T[:], hg[:], start=True, stop=True)
            fgf = small.tile([C, 1], f32)
            nc.vector.tensor_scalar(fgf[:], pg2[:], bg2_v[:], None, Alu.add)

            # ---------------- S2 pass A -----------------
            idx_all = big.tile([128, NB * K], u16)
            for b in range(NB):
                negd = pa.tile([128, N], f32, tag="negd")
                for j in range(6):
                    pd = ps(128, 512)
                    nc.tensor.matmul(pd[:], A1_m[:, 128 * b:128 * (b + 1)],
                                     B1_m[:, 512 * j:512 * (j + 1)],
                                     start=True, stop=False)
                    nc.tensor.matmul(pd[:], A2_m[:, 128 * b:128 * (b + 1)],
                                     B2_m[:, 512 * j:512 * (j + 1)],
                                     start=False, stop=True)
                    nc.scalar.copy(negd[:, 512 * j:512 * (j + 1)], pd[:])
                if debug and b == 0:
                    nc.sync.dma_start(AP(dbg_ext["d_negd"], 0, [[N, 128], [1, N]]),
                                      negd[:])
                v16 = small.tile([128, 16], f32, tag="v16")
                nc.vector.max(v16[:, 0:8], negd[:])
                nc.vector.max_index(idx_all[:, K * b:K * b + 8], v16[:, 0:8], negd[:])
                mrout = pa.tile([128, N], f32, tag="mrout")
                nc.vector.match_replace(mrout[:], v16[:, 0:8], negd[:], -1e30)
                nc.vector.max(v16[:, 8:16], mrout[:])
                nc.vector.max_index(idx_all[:, K * b + 8:K * b + 16],
                                    v16[:, 8:16], mrout[:])

            # ---------------- S3 gather prep -----------------
            # Stage idx to DRAM so that each gather call (bgrp, k) reads a
            # contiguous wrapped [16, 32] block:
            #   dram2 addr = ((b//BT)*K + k)*512 + (p%16)*32 + (b%BT)*8 + p//16
            NBG = NB // BT
            idx_dram = dram.tile([NBG * K * 512], i16)
            for ph in range(8):
                for bg in range(NBG):
                    nc.sync.dma_start(
                        AP(idx_dram.tensor, bg * 512 * K + ph,
                           [[32, 16], [8, BT], [512, K]]),
                        idx_all[16 * ph:16 * (ph + 1),
                                bg * BT * K:(bg + 1) * BT * K].bitcast(i16)
                        .rearrange("p (bl k) -> p bl k", k=K))
            idx_wrap = small.tile([128, NBG * K, 32], i16)
            for g in range(8):
                nc.sync.dma_start(
                    idx_wrap[16 * g:16 * (g + 1), :, :],
                    AP(idx_dram.tensor, 0, [[32, 16], [512, NBG * K], [1, 32]]))

            if debug:
                idxf = small.tile([128, NB * K], f32, tag="idxf")
                nc.vector.tensor_copy(idxf[:], idx_all[:])
                nc.sync.dma_start(AP(dbg_ext["d_idx"], 0, [[NB * K, 128], [1, NB * K]]),
                                  idxf[:])

            # ---------------- S4 pass C -----------------
            ctx.close_pa()
            agg = big.tile([128, NB, C], f32)
            dp = small.tile([128, NB, 2], f32)
            if debug:
                dnum = big.tile([128, NB, K], f32, tag="dnum")
                duw = big.tile([128, NB, K], f32, tag="duw")
            pc = ctx.enter_pc(tc.tile_pool(name="passc", bufs=2))
            pcw = ctx.enter_pc(tc.tile_pool(name="passcw", bufs=1))
            for s in range(NSL):
                b0 = BT * s
                fnei = pc.tile([128, BT, K, 128], f32, tag="fnei")
                for kq in range(K):
                    gth = pc.tile([128, BT * 128], f32, tag="gth")
                    nc.gpsimd.ap_gather(gth[:], fxT[:],
                                        idx_wrap[:, s * K + kq:s * K + kq + 1, :].rearrange("p a q -> p (a q)"),
                                        channels=128, num_elems=N, d=1,
                                        num_idxs=BT * 128)
                    ptg = ps(128, BT * 128)
                    for q in range(BT):
                        nc.tensor.matmul(ptg[:, 128 * q:128 * (q + 1)],
                                         gth[:, 128 * q:128 * (q + 1)],
                                         ident[:, :], is_transpose=True)
                    nc.scalar.copy(fnei[:, :, kq:kq + 1, :], ptg[:])
                xyz_nei = fnei[:, :, :, C:C + 3]
                f_nei = fnei[:, :, :, 0:C]
                xsl = xyz_sb[:, b0:b0 + BT, :]
                dxyz = pcw.tile([128, BT, K, 3], f32, tag="dxyz")
                nc.vector.tensor_tensor(
                    dxyz[:], xyz_nei,
                    xsl.unsqueeze(2).broadcast_to((128, BT, K, 3)), Alu.subtract)
                t3 = pcw.tile([128, BT, K, 3], f32, tag="t3")
                nc.vector.tensor_tensor(t3[:], dxyz[:], dxyz[:], Alu.mult)
                d2k = pcw.tile([128, BT, K], f32, tag="d2k")
                nc.vector.tensor_reduce(d2k[:], t3[:], axis=AX.X, op=Alu.add)
                sqd = pcw.tile([128, BT, K], f32, tag="sqd")
                nc.scalar.activation(sqd[:], d2k[:], Act.Sqrt,
                                     bias=zero128[:], scale=1.0)
                den = pcw.tile([128, BT, K], f32, tag="den")
                nc.vector.tensor_scalar(den[:], sqd[:], 1e-6, None, Alu.max)
                rden = pcw.tile([128, BT, K], f32, tag="rden")
                nc.vector.reciprocal(rden[:], den[:])
                nc.vector.tensor_tensor(
                    t3[:], dxyz[:],
                    vhat[:, b0:b0 + BT, :].unsqueeze(2).broadcast_to((128, BT, K, 3)),
                    Alu.mult)
                numv = pcw.tile([128, BT, K], f32, tag="numv")
                nc.vector.tensor_reduce(numv[:], t3[:], axis=AX.X, op=Alu.add)
                if debug:
                    nc.vector.tensor_copy(dnum[:, b0:b0 + BT, :], numv[:])
                ek = pcw.tile([128, BT, K], f32, tag="ek")
                nc.vector.tensor_tensor(ek[:], numv[:], rden[:], Alu.mult)
                nc.scalar.activation(ek[:], ek[:], Act.Exp,
                                     bias=zero128[:], scale=1.0 / TAU)
                se = pcw.tile([128, BT, 1], f32, tag="se")
                nc.vector.tensor_reduce(se[:], ek[:], axis=AX.X, op=Alu.add)
                rse = pcw.tile([128, BT, 1], f32, tag="rse")
                nc.vector.reciprocal(rse[:], se[:])
                uw = pcw.tile([128, BT, K], f32, tag="uw")
                nc.vector.tensor_tensor(uw[:], ek[:],
                                        rse[:].broadcast_to((128, BT, K)), Alu.mult)
                nc.vector.tensor_tensor(
                    uw[:], uw[:],
                    de16[:, b0:b0 + BT, :].broadcast_to((128, BT, K)), Alu.add)
                if debug:
                    nc.vector.tensor_copy(duw[:, b0:b0 + BT, :], uw[:])
                    if s == 0:
                        nc.sync.dma_start(
                            AP(dbg_ext["d_fnei"], 0,
                               [[BT * K * 128, 128], [1, BT * K * 128]]), fnei[:])
                prod = pcw.tile([128, BT, K, C], f32, tag="prod")
                nc.vector.tensor_tensor(
                    prod[:], f_nei,
                    uw[:].unsqueeze(3).broadcast_to((128, BT, K, C)), Alu.mult)
                pv = prod[:].rearrange("p b k c -> p b k c")
                s1 = pcw.tile([128, BT, 8, C], f32, tag="s1")
                prodv = prod[:].rearrange("p b (k2 two) c -> p b k2 (two c)", two=2)
                nc.vector.tensor_tensor(s1[:], prodv[:, :, :, 0:C],
                                        prodv[:, :, :, C:2 * C], Alu.add)
                s2 = pcw.tile([128, BT, 4, C], f32, tag="s2")
                s1v = s1[:].rearrange("p b (k2 two) c -> p b k2 (two c)", two=2)
                nc.vector.tensor_tensor(s2[:], s1v[:, :, :, 0:C],
                                        s1v[:, :, :, C:2 * C], Alu.add)
                s3 = pcw.tile([128, BT, 2, C], f32, tag="s3")
                s2v = s2[:].rearrange("p b (k2 two) c -> p b k2 (two c)", two=2)
                nc.vector.tensor_tensor(s3[:], s2v[:, :, :, 0:C],
                                        s2v[:, :, :, C:2 * C], Alu.add)
                s3v = s3[:].rearrange("p b (one two) c -> p b one (two c)", two=2)
                nc.vector.tensor_tensor(agg[:, b0:b0 + BT, :], s3v[:, :, :, 0:C],
                                        s3v[:, :, :, C:2 * C], Alu.add)
                # dist stats
                ndsl = dp[:, b0:b0 + BT, 0:1]
                nvsl = dp[:, b0:b0 + BT, 1:2]
                nc.vector.tensor_reduce(ndsl, sqd[:], axis=AX.X, op=Alu.add)
                nc.vector.tensor_scalar(ndsl, ndsl, 1.0 / K, None, Alu.mult)
                d2m = pcw.tile([128, BT, 1], f32, tag="d2m")
                nc.vector.tensor_reduce(d2m[:], d2k[:], axis=AX.X, op=Alu.add)
                nc.vector.tensor_scalar(d2m[:], d2m[:], 1.0 / K, None, Alu.mult)
                nd2 = pcw.tile([128, BT, 1], f32, tag="nd2")
                nc.vector.tensor_tensor(nd2[:], ndsl, ndsl, Alu.mult)
                nc.vector.tensor_tensor(nvsl, d2m[:], nd2[:], Alu.subtract)

            if debug:
                nc.sync.dma_start(AP(dbg_ext["d_agg"], 0, [[NB * C, 128], [1, NB * C]]),
                                  agg[:])
                nc.sync.dma_start(AP(dbg_ext["d_de"], 0, [[NB, 128], [1, NB]]), de[:])
                nc.sync.dma_start(AP(dbg_ext["d_dist"], 0, [[NB * 2, 128], [1, NB * 2]]),
                                  dp[:])
                nc.sync.dma_start(AP(dbg_ext["d_heads"], 0, [[N, 5], [1, N]]),
                                  headsT[:])
                nc.sync.dma_start(AP(dbg_ext["d_num"], 0, [[NB * K, 128], [1, NB * K]]), dnum[:])
                nc.sync.dma_start(AP(dbg_ext["d_uw"], 0, [[NB * K, 128], [1, NB * K]]), duw[:])
                nc.sync.dma_start(AP(dbg_ext["d_vhat"], 0, [[NB * 3, 128], [1, NB * 3]]), vhat[:])

            # ---------------- S5 reaction + BN -----------------
            ctx.close_pc()
            late = ctx.enter_context(tc.tile_pool(name="late", bufs=1))
            distT = late.tile([2, N], f32)
            for j in range(6):
                ptd = ps(2, 512)
                for q in range(4):
                    b = 4 * j + q
                    nc.tensor.matmul(ptd[:, 128 * q:128 * (q + 1)],
                                     dp[:, b:b + 1, :], ident[:, :],
                                     is_transpose=True)
                nc.vector.tensor_copy(distT[:, 512 * j:512 * (j + 1)], ptd[:])

            x_sb = late.tile([C, N], f32)
            xs6 = small.tile([C, 6], f32)
            x2s6 = small.tile([C, 6], f32)
            scr = late.tile([C, N], f32)
            for j in range(6):
                sl = slice(512 * j, 512 * (j + 1))
                px = ps(C, 512)
                nc.tensor.matmul(px[:], Wr1fT[:], fT[:, sl], start=True, stop=False)
                nc.tensor.matmul(px[:], Wr1flT[:], headsT[0:3, sl],
                                 start=False, stop=False)
                nc.tensor.matmul(px[:], Wr1dT[:], distT[:, sl],
                                 start=False, stop=True)
                nc.scalar.activation(x_sb[:, sl], px[:], Act.Copy, bias=0.0,
                                     scale=1.0, accum_out=xs6[:, j:j + 1])
                nc.scalar.activation(scr[:, sl], x_sb[:, sl], Act.Square,
                                     bias=zero128[0:C, :], scale=1.0,
                                     accum_out=x2s6[:, j:j + 1])
            bn_loc = small.tile([C, 2], f32)
            nc.vector.tensor_reduce(bn_loc[:, 0:1], xs6[:], axis=AX.X, op=Alu.add)
            nc.vector.tensor_reduce(bn_loc[:, 1:2], x2s6[:], axis=AX.X, op=Alu.add)
            bn_in = dram.tile([C, 2], f32)
            bn_out = dram.tile([C, 2], f32)
            nc.sync.dma_start(bn_in[:], bn_loc[:])
            nc.gpsimd.collective_compute(
                "AllReduce", Alu.add, replica_groups=[list(range(NCORES))],
                ins=[bn_in[:].opt()], outs=[bn_out[:].opt()])
            bn_g = small.tile([C, 2], f32)
            nc.sync.dma_start(bn_g[:], bn_out[:])
            Mtot = float(NCORES * N)
            mu = small.tile([C, 1], f32)
            nc.vector.tensor_scalar(mu[:], bn_g[:, 0:1], 1.0 / Mtot, None, Alu.mult)
            var = small.tile([C, 1], f32)
            nc.vector.tensor_scalar(var[:], bn_g[:, 1:2], 1.0 / Mtot, None, Alu.mult)
            mu2 = small.tile([C, 1], f32)
            nc.vector.tensor_tensor(mu2[:], mu[:], mu[:], Alu.mult)
            nc.vector.tensor_tensor(var[:], var[:], mu2[:], Alu.subtract)
            nc.vector.tensor_scalar(var[:], var[:], BN_EPS, None, Alu.add)
            nc.scalar.activation(var[:], var[:], Act.Sqrt,
                                 bias=zero128[0:C, :], scale=1.0)
            rstd = small.tile([C, 1], f32)
            nc.vector.reciprocal(rstd[:], var[:])
            s_vec = small.tile([C, 1], f32)
            nc.vector.tensor_tensor(s_vec[:], gamma_v[:], rstd[:], Alu.mult)
            b_vec = small.tile([C, 1], f32)
            nc.vector.tensor_tensor(b_vec[:], mu[:], s_vec[:], Alu.mult)
            nc.vector.tensor_tensor(b_vec[:], beta_v[:], b_vec[:], Alu.subtract)
            if debug:
                nc.sync.dma_start(AP(dbg_ext["d_bn"], 0, [[4, C], [1, 2]]), bn_g[:])
                nc.sync.dma_start(AP(dbg_ext["d_bn"], 2, [[4, C], [1, 1]]), mu[:])
                nc.sync.dma_start(AP(dbg_ext["d_bn"], 3, [[4, C], [1, 1]]), var[:])

            nc.scalar.activation(x_sb[:], x_sb[:], Act.Relu,
                                 bias=b_vec[:], scale=s_vec[:])
            nc.vector.tensor_scalar(gateT[:], gateT[:], fgf[:], None, Alu.mult)
            for j in range(6):
                sl = slice(512 * j, 512 * (j + 1))
                pr = ps(C, 512)
                nc.tensor.matmul(pr[:], Wr2T[:], x_sb[:, sl], start=True, stop=True)
                nc.scalar.copy(scr[:, sl], pr[:])
            nc.vector.tensor_scalar(scr[:], scr[:], br2_v[:], None, Alu.add)
            nc.vector.tensor_tensor(gateT[:], gateT[:], scr[:], Alu.add)

            TRp = late.tile([128, NB, C], f32)
            for j in range(3):
                pt = ps(128, 512)
                for q in range(8):
                    b = 8 * j + q
                    nc.tensor.matmul(pt[:, C * q:C * (q + 1)],
                                     gateT[:, 128 * b:128 * (b + 1)],
                                     ident[0:C, 0:C], is_transpose=True)
                nc.scalar.copy(TRp[:, 8 * j:8 * (j + 1), :], pt[:])

            # ---------------- S6 final combine -----------------
            de1 = small.tile([128, NB, 1], f32)
            nc.vector.tensor_scalar(de1[:], de[:], 1.0, None, Alu.add)
            out_sb = late.tile([128, NB, C], f32)
            nc.vector.tensor_tensor(out_sb[:], f_sb[:],
                                    de1[:].broadcast_to((128, NB, C)), Alu.mult)
            nc.vector.tensor_tensor(agg[:], agg[:], out_sb[:], Alu.subtract)
            nc.vector.tensor_tensor(agg[:], agg[:], TRp[:], Alu.add)
            nc.vector.scalar_tensor_tensor(out_sb[:], agg[:], dtv[:], f_sb[:],
                                           Alu.mult, Alu.add)
            nc.sync.dma_start(AP(out_ext, 0, [[C, 128], [128 * C, NB], [1, C]]),
                              out_sb[:])

    nc.compile()
    return nc


@functools.cache
def _get_nc(debug=False):
    return _build(debug=debug)


def _run(nc, inputs, trace=False):
    from concourse.bass_utils import run_bass_kernel_spmd
    f_seq = np.ascontiguousarray(np.asarray(inputs["f_seq"], dtype=np.float32))
    xyz = np.ascontiguousarray(np.asarray(inputs["xyz"], dtype=np.float32))
    in_maps = []
    for core in range(NCORES):
        b, l = divmod(core, L)
        m = {"f": f_seq[b, l], "xyz": xyz[b, l]}
        for k in WEIGHT_NAMES:
            m[k] = np.ascontiguousarray(
                np.asarray(inputs[k], dtype=np.float32).reshape(-1))
        in_maps.append(m)
    return run_bass_kernel_spmd(nc, in_maps, core_ids=list(range(NCORES)),
                                trace=trace)


def kernel(**inputs):
    nc = _get_nc()
    res = _run(nc, inputs)
    out = np.stack([np.asarray(res.results[i]["out"]) for i in range(NCORES)])
    return out.reshape(B, L, N, C).astype(np.float32)


# revision 23
# speedup vs baseline: 617.4365x; 617.4365x over previous
"""Trainium2 Bass kernel for ADRiverDynamics (gnn_message_passing).

8 independent point clouds (B*L=8), one per NeuronCore (pure data parallel),
plus one tiny AllReduce for global BatchNorm statistics.

Per-core pipeline (cloud of N=3072 points, C=64 channels, K=16 neighbors):
  S0  load f/xyz, weights; build combined DRAM rows [f|xyz|pad] for gathers
  S1  PE transposes (fT, xyzT->A/B), head convs (flow/diff/unc), gate conv
  S2  pass A: negD = -dist^2 via matmul (two accumulating calls that bit-match
      the reference's d2 formula); per-row top-16 of 3072 via 8-way segmented
      max8 + max_index, merged with match_replace, index indirection resolved
      with two gpsimd local_scatter ops (rank trick)
  S3  neighbor f/xyz gather: gpsimd ap_gather of fxT columns (idx staged via a
      DRAM round-trip into the per-core wrapped layout) + PE transposes back
      to point-major layout
  S4  pass C: K-dense math (cos/softmax weights), fused weighted aggregation
  S5  reaction conv + global-batch BN (AllReduce) + relu + conv
  S6  combine: out = f + dt*(adv + diff + reac)
"""
import functools
import numpy as np

B, L, N, C, K = 2, 4, 3072, 64, 16
NB = N // 128          # 24 point blocks
TAU = 0.15
BN_EPS = 1e-5
NCORES = 8
BT = 4                 # blocks per pass-C slice
NSL = NB // BT         # pass-C slices

WEIGHT_NAMES = ["Wf", "bf", "Wd", "bd", "Wu", "bu", "Wg1", "bg1", "Wg2", "bg2",
                "Wgate", "bgate", "Wr1", "br1", "gamma", "beta", "Wr2", "br2",
                "log_dt"]


def _build(debug=False, nocol=False):
    import contextlib
    from concourse import bacc
    import concourse.bass as bass
    import concourse.tile as tile
    import concourse.mybir as mybir
    from concourse import masks

    f32 = mybir.dt.float32
    u16 = mybir.dt.uint16
    i16 = mybir.dt.int16
    Alu = mybir.AluOpType
    Act = mybir.ActivationFunctionType
    AX = mybir.AxisListType
    AP = bass.AP

    nc = bacc.Bacc("TRN2", target_bir_lowering=False, debug=False,
                   num_devices=NCORES)

    f_ext = nc.dram_tensor("f", [N, C], f32, kind="ExternalInput")
    xyz_ext = nc.dram_tensor("xyz", [N, 3], f32, kind="ExternalInput")
    wshapes = {"Wf": [3, C], "bf": [3], "Wd": [1, C], "bd": [1], "Wu": [1, C],
               "bu": [1], "Wg1": [C, 3], "bg1": [C], "Wg2": [C, C], "bg2": [C],
               "Wgate": [C, C], "bgate": [C], "Wr1": [C, C + 5], "br1": [C],
               "gamma": [C], "beta": [C], "Wr2": [C, C], "br2": [C],
               "log_dt": [1]}
    w_ext = {k: nc.dram_tensor(k, shp, f32, kind="ExternalInput")
             for k, shp in wshapes.items()}
    out_ext = nc.dram_tensor("out", [N, C], f32, kind="ExternalOutput")
    dbg_ext = {}
    if debug:
        for k, shp in {"d_idx": [128, NB * K], "d_agg": [128, NB * C],
                       "d_de": [128, NB], "d_dist": [128, NB * 2],
                       "d_bn": [C, 4], "d_heads": [5, N],
                       "d_negd": [128, N], "d_num": [128, NB * K],
                       "d_uw": [128, NB * K], "d_fnei": [128, BT * K * 128],
                       "d_vhat": [128, NB * 3]}.items():
            dbg_ext[k] = nc.dram_tensor(k, shp, f32, kind="ExternalOutput")

    with tile.TileContext(nc) as tc:
        class _Stacks(contextlib.ExitStack):
            def __init__(self):
                super().__init__()
                self._pa = contextlib.ExitStack()
                self._pc = contextlib.ExitStack()
            def enter_pa(self, cm):
                return self._pa.enter_context(cm)
            def enter_pc(self, cm):
                return self._pc.enter_context(cm)
            def close_pa(self):
                self._pa.close()
            def close_pc(self):
                self._pc.close()
            def __exit__(self, *a):
                self._pc.close()
                self._pa.close()
                return super().__exit__(*a)
        ctx = _Stacks()
        with ctx:
            cpool = ctx.enter_context(tc.tile_pool(name="consts", bufs=1))
            big = ctx.enter_context(tc.tile_pool(name="big", bufs=1))
            dram = ctx.enter_context(tc.tile_pool(name="dram", bufs=1, space="DRAM"))
            psum = ctx.enter_context(tc.tile_pool(name="psum", bufs=4, space="PSUM"))
            small = ctx.enter_context(tc.tile_pool(name="small", bufs=1))

            def ps(p, fr):
                return psum.tile([p, fr], f32, tag="ps", name="pst")

            # ---------------- constants / weights -----------------
            ident = cpool.tile([128, 128], f32)
            masks.make_identity(nc, ident[:])

            WhT = cpool.tile([C, 5], f32)
            nc.sync.dma_start(WhT[:, 0:3], AP(w_ext["Wf"], 0, [[1, C], [C, 3]]))
            nc.sync.dma_start(WhT[:, 3:4], AP(w_ext["Wd"], 0, [[1, C], [C, 1]]))
            nc.sync.dma_start(WhT[:, 4:5], AP(w_ext["Wu"], 0, [[1, C], [C, 1]]))
            bhead = cpool.tile([5, 1], f32)
            nc.sync.dma_start(bhead[0:3, :], AP(w_ext["bf"], 0, [[1, 3], [1, 1]]))
            nc.sync.dma_start(bhead[3:4, :], AP(w_ext["bd"], 0, [[1, 1], [1, 1]]))
            nc.sync.dma_start(bhead[4:5, :], AP(w_ext["bu"], 0, [[1, 1], [1, 1]]))

            WgateT = cpool.tile([C, C], f32)
            nc.sync.dma_start(WgateT[:], AP(w_ext["Wgate"], 0, [[1, C], [C, C]]))
            Wg1T = cpool.tile([3, C], f32)
            nc.sync.dma_start(Wg1T[:], AP(w_ext["Wg1"], 0, [[1, 3], [3, C]]))
            Wg2T = cpool.tile([C, C], f32)
            nc.sync.dma_start(Wg2T[:], AP(w_ext["Wg2"], 0, [[1, C], [C, C]]))
            Wr1fT = cpool.tile([C, C], f32)
            nc.sync.dma_start(Wr1fT[:], AP(w_ext["Wr1"], 0, [[1, C], [C + 5, C]]))
            Wr1flT = cpool.tile([3, C], f32)
            nc.sync.dma_start(Wr1flT[:], AP(w_ext["Wr1"], C, [[1, 3], [C + 5, C]]))
            Wr1dT = cpool.tile([2, C], f32)
            nc.sync.dma_start(Wr1dT[:], AP(w_ext["Wr1"], C + 3, [[1, 2], [C + 5, C]]))
            Wr2T = cpool.tile([C, C], f32)
            nc.sync.dma_start(Wr2T[:], AP(w_ext["Wr2"], 0, [[1, C], [C, C]]))

            def vec_col(name):
                t = cpool.tile([C, 1], f32, tag=name, name=name + "_v")
                nc.sync.dma_start(t[:], AP(w_ext[name], 0, [[1, C], [1, 1]]))
                return t
            bgate_v = vec_col("bgate")
            bg1_v = vec_col("bg1")
            bg2_v = vec_col("bg2")
            br2_v = vec_col("br2")
            gamma_v = vec_col("gamma")
            beta_v = vec_col("beta")

            zero128 = cpool.tile([128, 1], f32)
            nc.vector.memset(zero128[:], 0.0)
            segb64u = cpool.tile([128, 64], u16)
            nc.gpsimd.iota(segb64u[:], pattern=[[384, 8], [0, 8]],
                           channel_multiplier=0)
            rank16 = cpool.tile([128, 16], i16)
            nc.gpsimd.iota(rank16[:], pattern=[[1, 16]], base=1,
                           channel_multiplier=0)

            dtv = cpool.tile([128, 1], f32)
            nc.sync.dma_start(dtv[:], AP(w_ext["log_dt"], 0, [[0, 128], [1, 1]]))
            nc.scalar.activation(dtv[:], dtv[:], Act.Exp, bias=zero128[:], scale=1.0)
            nc.vector.tensor_scalar(dtv[:], dtv[:], 1e-4, 10.0, Alu.max, Alu.min)

            # ---------------- S0 loads -----------------
            f_sb = big.tile([128, NB, C], f32)
            nc.sync.dma_start(f_sb[:], AP(f_ext, 0, [[C, 128], [128 * C, NB], [1, C]]))
            xyz_sb = big.tile([128, NB, 3], f32)
            nc.sync.dma_start(xyz_sb[:], AP(xyz_ext, 0, [[3, 128], [128 * 3, NB], [1, 3]]))


            # ---------------- S1 transposes + convs -----------------
            fxT = big.tile([128, N], f32)
            fT = fxT[0:C, :]
            for j in range(6):
                pt = ps(C, 512)
                for q in range(4):
                    b = 4 * j + q
                    nc.tensor.matmul(pt[:, 128 * q:128 * (q + 1)],
                                     f_sb[:, b:b + 1, :], ident[:, :],
                                     is_transpose=True)
                nc.scalar.copy(fxT[0:C, 512 * j:512 * (j + 1)], pt[:])

            pa = ctx.enter_pa(tc.tile_pool(name="passa", bufs=2))
            A1_m = pa.tile([3, N], f32, tag="A1_m", bufs=1)   # 2x
            B1_m = pa.tile([3, N], f32, tag="B1_m", bufs=1)   # x
            A2_m = pa.tile([2, N], f32, tag="A2_m", bufs=1)   # [-sq; -1]
            B2_m = pa.tile([2, N], f32, tag="B2_m", bufs=1)   # [1; sq]
            nc.vector.memset(A2_m[:], -1.0)   # row 1 keeps -1
            nc.vector.memset(B2_m[:], 1.0)    # row 0 keeps +1
            for j in range(6):
                pt = ps(3, 512)
                for q in range(4):
                    b = 4 * j + q
                    nc.tensor.matmul(pt[:, 128 * q:128 * (q + 1)],
                                     xyz_sb[:, b:b + 1, :], ident[:, :],
                                     is_transpose=True)
                nc.scalar.mul(A1_m[:, 512 * j:512 * (j + 1)], pt[:], 2.0)
                nc.vector.tensor_copy(B1_m[:, 512 * j:512 * (j + 1)], pt[:])
                nc.scalar.copy(fxT[C:C + 3, 512 * j:512 * (j + 1)], pt[:])

            xyz2 = small.tile([128, NB, 3], f32)
            nc.vector.tensor_tensor(xyz2[:], xyz_sb[:], xyz_sb[:], Alu.mult)
            sq_p = small.tile([128, NB, 1], f32)
            nc.vector.tensor_reduce(sq_p[:], xyz2[:], axis=AX.X, op=Alu.add)
            sqn_p = small.tile([128, NB, 1], f32)
            nc.vector.tensor_scalar(sqn_p[:], sq_p[:], -1.0, None, Alu.mult)
            pt = ps(NB, 128)
            nc.tensor.matmul(pt[:], sq_p[:], ident[:, :], is_transpose=True)
            sq24 = small.tile([NB, 128], f32)
            nc.vector.tensor_copy(sq24[:], pt[:])
            pt = ps(NB, 128)
            nc.tensor.matmul(pt[:], sqn_p[:], ident[:, :], is_transpose=True)
            sqn24 = small.tile([NB, 128], f32)
            nc.vector.tensor_copy(sqn24[:], pt[:])
            nc.sync.dma_start(B2_m[1:2, :], sq24[:])
            nc.sync.dma_start(A2_m[0:1, :], sqn24[:])

            headsT = big.tile([5, N], f32)
            gateT = big.tile([C, N], f32)
            for j in range(6):
                sl = slice(512 * j, 512 * (j + 1))
                ph = ps(5, 512)
                nc.tensor.matmul(ph[:], WhT[:], fT[:, sl], start=True, stop=True)
                nc.vector.tensor_scalar(headsT[:, sl], ph[:], bhead[:], None, Alu.add)
                pg = ps(C, 512)
                nc.tensor.matmul(pg[:], WgateT[:], fT[:, sl], start=True, stop=True)
                nc.scalar.activation(gateT[:, sl], pg[:], Act.Sigmoid,
                                     bias=bgate_v[:], scale=1.0)

            hp = small.tile([128, NB, 5], f32)
            pt5 = ps(128, NB * 5)
            for b in range(NB):
                nc.tensor.matmul(pt5[:, 5 * b:5 * (b + 1)],
                                 headsT[:, 128 * b:128 * (b + 1)], ident[0:5, 0:5],
                                 is_transpose=True)
            nc.vector.tensor_copy(hp[:], pt5[:])

            flow_p = hp[:, :, 0:3]
            de = small.tile([128, NB, 1], f32)
            tmp_b = small.tile([128, NB, 1], f32)
            nc.scalar.activation(tmp_b[:], hp[:, :, 3:4], Act.Exp,
                                 bias=zero128[:], scale=1.0)
            nc.vector.tensor_scalar(tmp_b[:], tmp_b[:], 1.0, None, Alu.add)
            nc.scalar.activation(tmp_b[:], tmp_b[:], Act.Ln,
                                 bias=zero128[:], scale=1.0)
            sgu = small.tile([128, NB, 1], f32)
            nc.scalar.activation(sgu[:], hp[:, :, 4:5], Act.Sigmoid,
                                 bias=zero128[:], scale=1.0)
            nc.vector.tensor_scalar(sgu[:], sgu[:], 1.0, None, Alu.add)
            nc.vector.tensor_tensor(de[:], tmp_b[:], sgu[:], Alu.mult)
            de16 = small.tile([128, NB, 1], f32)
            nc.vector.tensor_scalar(de16[:], de[:], 1.0 / K, None, Alu.mult)

            fl2 = small.tile([128, NB, 3], f32)
            nc.vector.tensor_tensor(fl2[:], flow_p, flow_p, Alu.mult)
            vn = small.tile([128, NB, 1], f32)
            nc.vector.tensor_reduce(vn[:], fl2[:], axis=AX.X, op=Alu.add)
            nc.scalar.activation(vn[:], vn[:], Act.Sqrt, bias=zero128[:], scale=1.0)
            nc.vector.tensor_scalar(vn[:], vn[:], 1e-6, None, Alu.max)
            rv = small.tile([128, NB, 1], f32)
            nc.vector.reciprocal(rv[:], vn[:])
            vhat = small.tile([128, NB, 3], f32)
            nc.vector.tensor_tensor(vhat[:], flow_p,
                                    rv[:].broadcast_to((128, NB, 3)), Alu.mult)

            fgm = small.tile([3, 1], f32)
            nc.vector.tensor_reduce(fgm[:], headsT[0:3, :], axis=AX.X, op=Alu.add)
            nc.vector.tensor_scalar(fgm[:], fgm[:], 1.0 / N, None, Alu.mult)
            pg1 = ps(C, 1)
            nc.tensor.matmul(pg1[:], Wg1T[:], fgm[:], start=True, stop=True)
            hg = small.tile([C, 1], f32)
            nc.scalar.activation(hg[:], pg1[:], Act.Relu, bias=bg1_v[:], scale=1.0)
            pg2 = ps(C, 1)
            nc.tensor.matmul(pg2[:], Wg2T[:], hg[:], start=True, stop=True)
            fgf = small.tile([C, 1], f32)
            nc.vector.tensor_scalar(fgf[:], pg2[:], bg2_v[:], None, Alu.add)

            # ---------------- S2 pass A -----------------
            idx_all = big.tile([128, NB * K], u16)
            for b in range(NB):
                negd = pa.tile([128, N], f32, tag="negd")
                for j in range(6):
                    pd = ps(128, 512)
                    nc.tensor.matmul(pd[:], A1_m[:, 128 * b:128 * (b + 1)],
                                     B1_m[:, 512 * j:512 * (j + 1)],
                                     start=True, stop=False)
                    nc.tensor.matmul(pd[:], A2_m[:, 128 * b:128 * (b + 1)],
                                     B2_m[:, 512 * j:512 * (j + 1)],
                                     start=False, stop=True)
                    nc.scalar.copy(negd[:, 512 * j:512 * (j + 1)], pd[:])
                if debug and b == 0:
                    nc.sync.dma_start(AP(dbg_ext["d_negd"], 0, [[N, 128], [1, N]]),
                                      negd[:])
                cand = small.tile([128, 64], f32, tag="cand")
                segloc = small.tile([128, 64], u16, tag="segloc")
                for s8 in range(8):
                    nc.vector.max(cand[:, 8 * s8:8 * (s8 + 1)],
                                  negd[:, 384 * s8:384 * (s8 + 1)])
                    nc.vector.max_index(segloc[:, 8 * s8:8 * (s8 + 1)],
                                        cand[:, 8 * s8:8 * (s8 + 1)],
                                        negd[:, 384 * s8:384 * (s8 + 1)])
                jc16 = small.tile([128, 64], u16, tag="jc16")
                nc.vector.tensor_tensor(jc16[:], segloc[:], segb64u[:], Alu.add)
                v16 = small.tile([128, 16], f32, tag="v16")
                mrc = small.tile([128, 64], f32, tag="mrc")
                cp16 = small.tile([128, 16], u16, tag="cp16")
                nc.vector.max(v16[:, 0:8], cand[:])
                nc.vector.max_index(cp16[:, 0:8], v16[:, 0:8], cand[:])
                nc.vector.match_replace(mrc[:], v16[:, 0:8], cand[:], -1e30)
                nc.vector.max(v16[:, 8:16], mrc[:])
                nc.vector.max_index(cp16[:, 8:16], v16[:, 8:16], mrc[:])
                rankmap = small.tile([128, 64], i16, tag="rankmap")
                nc.gpsimd.local_scatter(rankmap[:], rank16[:],
                                        cp16[:].bitcast(i16),
                                        channels=128, num_elems=64, num_idxs=16)
                nc.vector.tensor_scalar(rankmap[:], rankmap[:], 1, None,
                                        Alu.subtract)
                nc.gpsimd.local_scatter(idx_all[:, K * b:K * (b + 1)].bitcast(i16),
                                        jc16[:].bitcast(i16), rankmap[:],
                                        channels=128, num_elems=16, num_idxs=64)

            # ---------------- S3 gather prep -----------------
            # Stage idx to DRAM so that each gather call (bgrp, k) reads a
            # contiguous wrapped [16, 32] block:
            #   dram2 addr = ((b//BT)*K + k)*512 + (p%16)*32 + (b%BT)*8 + p//16
            NBG = NB // BT
            idx_dram = dram.tile([NBG * K * 512], i16)
            for ph in range(8):
                for bg in range(NBG):
                    nc.sync.dma_start(
                        AP(idx_dram.tensor, bg * 512 * K + ph,
                           [[32, 16], [8, BT], [512, K]]),
                        idx_all[16 * ph:16 * (ph + 1),
                                bg * BT * K:(bg + 1) * BT * K].bitcast(i16)
                        .rearrange("p (bl k) -> p bl k", k=K))
            idx_wrap = small.tile([128, NBG * K, 32], i16)
            for g in range(8):
                nc.sync.dma_start(
                    idx_wrap[16 * g:16 * (g + 1), :, :],
                    AP(idx_dram.tensor, 0, [[32, 16], [512, NBG * K], [1, 32]]))

            if debug:
                idxf = small.tile([128, NB * K], f32, tag="idxf")
                nc.vector.tensor_copy(idxf[:], idx_all[:])
                nc.sync.dma_start(AP(dbg_ext["d_idx"], 0, [[NB * K, 128], [1, NB * K]]),
                                  idxf[:])

            # ---------------- S4 pass C -----------------
            ctx.close_pa()
            agg = big.tile([128, NB, C], f32)
            dp = small.tile([128, NB, 2], f32)
            if debug:
                dnum = big.tile([128, NB, K], f32, tag="dnum")
                duw = big.tile([128, NB, K], f32, tag="duw")
            pc = ctx.enter_pc(tc.tile_pool(name="passc", bufs=2))
            pcw = ctx.enter_pc(tc.tile_pool(name="passcw", bufs=1))
            for s in range(NSL):
                b0 = BT * s
                fnei = pc.tile([128, BT, K, 128], f32, tag="fnei")
                for kq in range(K):
                    gth = pc.tile([128, BT * 128], f32, tag="gth")
                    nc.gpsimd.ap_gather(gth[:], fxT[:],
                                        idx_wrap[:, s * K + kq:s * K + kq + 1, :].rearrange("p a q -> p (a q)"),
                                        channels=128, num_elems=N, d=1,
                                        num_idxs=BT * 128)
                    ptg = ps(128, BT * 128)
                    for q in range(BT):
                        nc.tensor.matmul(ptg[:, 128 * q:128 * (q + 1)],
                                         gth[:, 128 * q:128 * (q + 1)],
                                         ident[:, :], is_transpose=True)
                    nc.scalar.copy(fnei[:, :, kq:kq + 1, :], ptg[:])
                xyz_nei = fnei[:, :, :, C:C + 3]
                f_nei = fnei[:, :, :, 0:C]
                xsl = xyz_sb[:, b0:b0 + BT, :]
                dxyz = pcw.tile([128, BT, K, 3], f32, tag="dxyz")
                nc.vector.tensor_tensor(
                    dxyz[:], xyz_nei,
                    xsl.unsqueeze(2).broadcast_to((128, BT, K, 3)), Alu.subtract)
                t3 = pcw.tile([128, BT, K, 3], f32, tag="t3")
                nc.vector.tensor_tensor(t3[:], dxyz[:], dxyz[:], Alu.mult)
                d2k = pcw.tile([128, BT, K], f32, tag="d2k")
                nc.vector.tensor_reduce(d2k[:], t3[:], axis=AX.X, op=Alu.add)
                sqd = pcw.tile([128, BT, K], f32, tag="sqd")
                nc.scalar.activation(sqd[:], d2k[:], Act.Sqrt,
                                     bias=zero128[:], scale=1.0)
                den = pcw.tile([128, BT, K], f32, tag="den")
                nc.vector.tensor_scalar(den[:], sqd[:], 1e-6, None, Alu.max)
                rden = pcw.tile([128, BT, K], f32, tag="rden")
                nc.vector.reciprocal(rden[:], den[:])
                nc.vector.tensor_tensor(
                    t3[:], dxyz[:],
                    vhat[:, b0:b0 + BT, :].unsqueeze(2).broadcast_to((128, BT, K, 3)),
                    Alu.mult)
                numv = pcw.tile([128, BT, K], f32, tag="numv")
                nc.vector.tensor_reduce(numv[:], t3[:], axis=AX.X, op=Alu.add)
                if debug:
                    nc.vector.tensor_copy(dnum[:, b0:b0 + BT, :], numv[:])
                ek = pcw.tile([128, BT, K], f32, tag="ek")
                nc.vector.tensor_tensor(ek[:], numv[:], rden[:], Alu.mult)
                nc.scalar.activation(ek[:], ek[:], Act.Exp,
                                     bias=zero128[:], scale=1.0 / TAU)
                se = pcw.tile([128, BT, 1], f32, tag="se")
                nc.vector.tensor_reduce(se[:], ek[:], axis=AX.X, op=Alu.add)
                rse = pcw.tile([128, BT, 1], f32, tag="rse")
                nc.vector.reciprocal(rse[:], se[:])
                uw = pcw.tile([128, BT, K], f32, tag="uw")
                nc.vector.tensor_tensor(uw[:], ek[:],
                                        rse[:].broadcast_to((128, BT, K)), Alu.mult)
                nc.vector.tensor_tensor(
                    uw[:], uw[:],
                    de16[:, b0:b0 + BT, :].broadcast_to((128, BT, K)), Alu.add)
                if debug:
                    nc.vector.tensor_copy(duw[:, b0:b0 + BT, :], uw[:])
                    if s == 0:
                        nc.sync.dma_start(
                            AP(dbg_ext["d_fnei"], 0,
                               [[BT * K * 128, 128], [1, BT * K * 128]]), fnei[:])
                prod = pcw.tile([128, BT, K, C], f32, tag="prod")
                nc.vector.tensor_tensor(
                    prod[:], f_nei,
                    uw[:].unsqueeze(3).broadcast_to((128, BT, K, C)), Alu.mult)
                pv = prod[:].rearrange("p b k c -> p b k c")
                s1 = pcw.tile([128, BT, 8, C], f32, tag="s1")
                prodv = prod[:].rearrange("p b (k2 two) c -> p b k2 (two c)", two=2)
                nc.vector.tensor_tensor(s1[:], prodv[:, :, :, 0:C],
                                        prodv[:, :, :, C:2 * C], Alu.add)
                s2 = pcw.tile([128, BT, 4, C], f32, tag="s2")
                s1v = s1[:].rearrange("p b (k2 two) c -> p b k2 (two c)", two=2)
                nc.vector.tensor_tensor(s2[:], s1v[:, :, :, 0:C],
                                        s1v[:, :, :, C:2 * C], Alu.add)
                s3 = pcw.tile([128, BT, 2, C], f32, tag="s3")
                s2v = s2[:].rearrange("p b (k2 two) c -> p b k2 (two c)", two=2)
                nc.vector.tensor_tensor(s3[:], s2v[:, :, :, 0:C],
                                        s2v[:, :, :, C:2 * C], Alu.add)
                s3v = s3[:].rearrange("p b (one two) c -> p b one (two c)", two=2)
                nc.vector.tensor_tensor(agg[:, b0:b0 + BT, :], s3v[:, :, :, 0:C],
                                        s3v[:, :, :, C:2 * C], Alu.add)
                # dist stats
                ndsl = dp[:, b0:b0 + BT, 0:1]
                nvsl = dp[:, b0:b0 + BT, 1:2]
                nc.vector.tensor_reduce(ndsl, sqd[:], axis=AX.X, op=Alu.add)
                nc.vector.tensor_scalar(ndsl, ndsl, 1.0 / K, None, Alu.mult)
                d2m = pcw.tile([128, BT, 1], f32, tag="d2m")
                nc.vector.tensor_reduce(d2m[:], d2k[:], axis=AX.X, op=Alu.add)
                nc.vector.tensor_scalar(d2m[:], d2m[:], 1.0 / K, None, Alu.mult)
                nd2 = pcw.tile([128, BT, 1], f32, tag="nd2")
                nc.vector.tensor_tensor(nd2[:], ndsl, ndsl, Alu.mult)
                nc.vector.tensor_tensor(nvsl, d2m[:], nd2[:], Alu.subtract)

            if debug:
                nc.sync.dma_start(AP(dbg_ext["d_agg"], 0, [[NB * C, 128], [1, NB * C]]),
                                  agg[:])
                nc.sync.dma_start(AP(dbg_ext["d_de"], 0, [[NB, 128], [1, NB]]), de[:])
                nc.sync.dma_start(AP(dbg_ext["d_dist"], 0, [[NB * 2, 128], [1, NB * 2]]),
                                  dp[:])
                nc.sync.dma_start(AP(dbg_ext["d_heads"], 0, [[N, 5], [1, N]]),
                                  headsT[:])
                nc.sync.dma_start(AP(dbg_ext["d_num"], 0, [[NB * K, 128], [1, NB * K]]), dnum[:])
                nc.sync.dma_start(AP(dbg_ext["d_uw"], 0, [[NB * K, 128], [1, NB * K]]), duw[:])
                nc.sync.dma_start(AP(dbg_ext["d_vhat"], 0, [[NB * 3, 128], [1, NB * 3]]), vhat[:])

            # ---------------- S5 reaction + BN -----------------
            ctx.close_pc()
            late = ctx.enter_context(tc.tile_pool(name="late", bufs=1))
            distT = late.tile([2, N], f32)
            for j in range(6):
                ptd = ps(2, 512)
                for q in range(4):
                    b = 4 * j + q
                    nc.tensor.matmul(ptd[:, 128 * q:128 * (q + 1)],
                                     dp[:, b:b + 1, :], ident[:, :],
                                     is_transpose=True)
                nc.vector.tensor_copy(distT[:, 512 * j:512 * (j + 1)], ptd[:])

            x_sb = late.tile([C, N], f32)
            xs6 = small.tile([C, 6], f32)
            x2s6 = small.tile([C, 6], f32)
            scr = late.tile([C, N], f32)
            for j in range(6):
                sl = slice(512 * j, 512 * (j + 1))
                px = ps(C, 512)
                nc.tensor.matmul(px[:], Wr1fT[:], fT[:, sl], start=True, stop=False)
                nc.tensor.matmul(px[:], Wr1flT[:], headsT[0:3, sl],
                                 start=False, stop=False)
                nc.tensor.matmul(px[:], Wr1dT[:], distT[:, sl],
                                 start=False, stop=True)
                nc.scalar.activation(x_sb[:, sl], px[:], Act.Copy, bias=0.0,
                                     scale=1.0, accum_out=xs6[:, j:j + 1])
                nc.scalar.activation(scr[:, sl], x_sb[:, sl], Act.Square,
                                     bias=zero128[0:C, :], scale=1.0,
                                     accum_out=x2s6[:, j:j + 1])
            bn_loc = small.tile([C, 2], f32)
            nc.vector.tensor_reduce(bn_loc[:, 0:1], xs6[:], axis=AX.X, op=Alu.add)
            nc.vector.tensor_reduce(bn_loc[:, 1:2], x2s6[:], axis=AX.X, op=Alu.add)
            bn_in = dram.tile([C, 2], f32)
            bn_out = dram.tile([C, 2], f32)
            bn_g = small.tile([C, 2], f32)
            if nocol:
                nc.vector.tensor_scalar(bn_g[:], bn_loc[:], float(NCORES), None,
                                        Alu.mult)
            else:
                nc.sync.dma_start(bn_in[:], bn_loc[:])
                nc.gpsimd.collective_compute(
                    "AllReduce", Alu.add, replica_groups=[list(range(NCORES))],
                    ins=[bn_in[:].opt()], outs=[bn_out[:].opt()])
                nc.sync.dma_start(bn_g[:], bn_out[:])
            Mtot = float(NCORES * N)
            mu = small.tile([C, 1], f32)
            nc.vector.tensor_scalar(mu[:], bn_g[:, 0:1], 1.0 / Mtot, None, Alu.mult)
            var = small.tile([C, 1], f32)
            nc.vector.tensor_scalar(var[:], bn_g[:, 1:2], 1.0 / Mtot, None, Alu.mult)
            mu2 = small.tile([C, 1], f32)
            nc.vector.tensor_tensor(mu2[:], mu[:], mu[:], Alu.mult)
            nc.vector.tensor_tensor(var[:], var[:], mu2[:], Alu.subtract)
            nc.vector.tensor_scalar(var[:], var[:], BN_EPS, None, Alu.add)
            nc.scalar.activation(var[:], var[:], Act.Sqrt,
                                 bias=zero128[0:C, :], scale=1.0)
            rstd = small.tile([C, 1], f32)
            nc.vector.reciprocal(rstd[:], var[:])
            s_vec = small.tile([C, 1], f32)
            nc.vector.tensor_tensor(s_vec[:], gamma_v[:], rstd[:], Alu.mult)
            b_vec = small.tile([C, 1], f32)
            nc.vector.tensor_tensor(b_vec[:], mu[:], s_vec[:], Alu.mult)
            nc.vector.tensor_tensor(b_vec[:], beta_v[:], b_vec[:], Alu.subtract)
            if debug:
                nc.sync.dma_start(AP(dbg_ext["d_bn"], 0, [[4, C], [1, 2]]), bn_g[:])
                nc.sync.dma_start(AP(dbg_ext["d_bn"], 2, [[4, C], [1, 1]]), mu[:])
                nc.sync.dma_start(AP(dbg_ext["d_bn"], 3, [[4, C], [1, 1]]), var[:])

            nc.scalar.activation(x_sb[:], x_sb[:], Act.Relu,
                                 bias=b_vec[:], scale=s_vec[:])
            nc.vector.tensor_scalar(gateT[:], gateT[:], fgf[:], None, Alu.mult)
            for j in range(6):
                sl = slice(512 * j, 512 * (j + 1))
                pr = ps(C, 512)
                nc.tensor.matmul(pr[:], Wr2T[:], x_sb[:, sl], start=True, stop=True)
                nc.scalar.copy(scr[:, sl], pr[:])
            nc.vector.tensor_scalar(scr[:], scr[:], br2_v[:], None, Alu.add)
            nc.vector.tensor_tensor(gateT[:], gateT[:], scr[:], Alu.add)

            TRp = late.tile([128, NB, C], f32)
            for j in range(3):
                pt = ps(128, 512)
                for q in range(8):
                    b = 8 * j + q
                    nc.tensor.matmul(pt[:, C * q:C * (q + 1)],
                                     gateT[:, 128 * b:128 * (b + 1)],
                                     ident[0:C, 0:C], is_transpose=True)
                nc.scalar.copy(TRp[:, 8 * j:8 * (j + 1), :], pt[:])

            # ---------------- S6 final combine -----------------
            de1 = small.tile([128, NB, 1], f32)
            nc.vector.tensor_scalar(de1[:], de[:], 1.0, None, Alu.add)
            out_sb = late.tile([128, NB, C], f32)
            nc.vector.tensor_tensor(out_sb[:], f_sb[:],
                                    de1[:].broadcast_to((128, NB, C)), Alu.mult)
            nc.vector.tensor_tensor(agg[:], agg[:], out_sb[:], Alu.subtract)
            nc.vector.tensor_tensor(agg[:], agg[:], TRp[:], Alu.add)
            nc.vector.scalar_tensor_tensor(out_sb[:], agg[:], dtv[:], f_sb[:],
                                           Alu.mult, Alu.add)
            nc.sync.dma_start(AP(out_ext, 0, [[C, 128], [128 * C, NB], [1, C]]),
                              out_sb[:])

    nc.compile()
    return nc


@functools.cache
def _get_nc(debug=False):
    return _build(debug=debug)


def _run(nc, inputs, trace=False):
    from concourse.bass_utils import run_bass_kernel_spmd
    f_seq = np.ascontiguousarray(np.asarray(inputs["f_seq"], dtype=np.float32))
    xyz = np.ascontiguousarray(np.asarray(inputs["xyz"], dtype=np.float32))
    in_maps = []
    for core in range(NCORES):
        b, l = divmod(core, L)
        m = {"f": f_seq[b, l], "xyz": xyz[b, l]}
        for k in WEIGHT_NAMES:
            m[k] = np.ascontiguousarray(
                np.asarray(inputs[k], dtype=np.float32).reshape(-1))
        in_maps.append(m)
    return run_bass_kernel_spmd(nc, in_maps, core_ids=list(range(NCORES)),
                                trace=trace)


def kernel(**inputs):
    nc = _get_nc()
    res = _run(nc, inputs)
    out = np.stack([np.asarray(res.results[i]["out"]) for i in range(NCORES)])
    return out.reshape(B, L, N, C).astype(np.float32)


# revision 36
# speedup vs baseline: 1134.8162x; 1.8379x over previous
"""Trainium2 Bass kernel for ADRiverDynamics (gnn_message_passing).

8 independent point clouds (B*L=8), one per NeuronCore (pure data parallel),
plus one tiny AllReduce for global BatchNorm statistics.

Per-core pipeline (cloud of N=3072 points, C=64 channels, K=16 neighbors):
  S0  load f/xyz, weights; build combined DRAM rows [f|xyz|pad] for gathers
  S1  PE transposes (fT, xyzT->A/B), head convs (flow/diff/unc), gate conv
  S2  pass A: negD = -dist^2 via matmul (two accumulating calls that bit-match
      the reference's d2 formula); per-row top-16 of 3072 via 8-way segmented
      max8 + max_index, merged with match_replace, index indirection resolved
      with two gpsimd local_scatter ops (rank trick)
  S3  neighbor f/xyz gather: gpsimd ap_gather of fxT columns (idx staged via a
      DRAM round-trip into the per-core wrapped layout) + PE transposes back
      to point-major layout
  S4  pass C: K-dense math (cos/softmax weights), fused weighted aggregation
  S5  reaction conv + global-batch BN (AllReduce) + relu + conv
  S6  combine: out = f + dt*(adv + diff + reac)
"""
import functools
import numpy as np

B, L, N, C, K = 2, 4, 3072, 64, 16
NB = N // 128          # 24 point blocks
TAU = 0.15
BN_EPS = 1e-5
NCORES = 8
BT = 4                 # blocks per pass-C slice
NSL = NB // BT         # pass-C slices

WEIGHT_NAMES = ["Wf", "bf", "Wd", "bd", "Wu", "bu", "Wg1", "bg1", "Wg2", "bg2",
                "Wgate", "bgate", "Wr1", "br1", "gamma", "beta", "Wr2", "br2",
                "log_dt"]


def _build(debug=False, nocol=False):
    import contextlib
    from concourse import bacc
    import concourse.bass as bass
    import concourse.tile as tile
    import concourse.mybir as mybir
    from concourse import masks

    f32 = mybir.dt.float32
    u16 = mybir.dt.uint16
    i16 = mybir.dt.int16
    Alu = mybir.AluOpType
    Act = mybir.ActivationFunctionType
    AX = mybir.AxisListType
    AP = bass.AP

    nc = bacc.Bacc("TRN2", target_bir_lowering=False, debug=False,
                   num_devices=NCORES)

    f_ext = nc.dram_tensor("f", [N, C], f32, kind="ExternalInput")
    xyz_ext = nc.dram_tensor("xyz", [N, 3], f32, kind="ExternalInput")
    wshapes = {"Wf": [3, C], "bf": [3], "Wd": [1, C], "bd": [1], "Wu": [1, C],
               "bu": [1], "Wg1": [C, 3], "bg1": [C], "Wg2": [C, C], "bg2": [C],
               "Wgate": [C, C], "bgate": [C], "Wr1": [C, C + 5], "br1": [C],
               "gamma": [C], "beta": [C], "Wr2": [C, C], "br2": [C],
               "log_dt": [1]}
    w_ext = {k: nc.dram_tensor(k, shp, f32, kind="ExternalInput")
             for k, shp in wshapes.items()}
    out_ext = nc.dram_tensor("out", [N, C], f32, kind="ExternalOutput")
    dbg_ext = {}
    if debug:
        for k, shp in {"d_idx": [128, NB * K], "d_agg": [128, NB * C],
                       "d_de": [128, NB], "d_dist": [128, NB * 2],
                       "d_bn": [C, 4], "d_heads": [5, N],
                       "d_negd": [128, N], "d_num": [128, NB * K],
                       "d_uw": [128, NB * K], "d_fnei": [128, BT * K * 80],
                       "d_vhat": [128, NB * 3]}.items():
            dbg_ext[k] = nc.dram_tensor(k, shp, f32, kind="ExternalOutput")

    with tile.TileContext(nc) as tc:
        class _Stacks(contextlib.ExitStack):
            def __init__(self):
                super().__init__()
                self._pa = contextlib.ExitStack()
                self._pc = contextlib.ExitStack()
            def enter_pa(self, cm):
                return self._pa.enter_context(cm)
            def enter_pc(self, cm):
                return self._pc.enter_context(cm)
            def close_pa(self):
                self._pa.close()
            def close_pc(self):
                self._pc.close()
            def __exit__(self, *a):
                self._pc.close()
                self._pa.close()
                return super().__exit__(*a)
        ctx = _Stacks()
        with ctx:
            cpool = ctx.enter_context(tc.tile_pool(name="consts", bufs=1))
            big = ctx.enter_context(tc.tile_pool(name="big", bufs=1))
            dram = ctx.enter_context(tc.tile_pool(name="dram", bufs=1, space="DRAM"))
            psum = ctx.enter_context(tc.tile_pool(name="psum", bufs=4, space="PSUM"))
            small = ctx.enter_context(tc.tile_pool(name="small", bufs=1))

            def ps(p, fr):
                return psum.tile([p, fr], f32, tag="ps", name="pst")

            # ---------------- constants / weights -----------------
            ident = cpool.tile([128, 128], f32)
            masks.make_identity(nc, ident[:])

            WhT = cpool.tile([C, 5], f32)
            nc.sync.dma_start(WhT[:, 0:3], AP(w_ext["Wf"], 0, [[1, C], [C, 3]]))
            nc.sync.dma_start(WhT[:, 3:4], AP(w_ext["Wd"], 0, [[1, C], [C, 1]]))
            nc.sync.dma_start(WhT[:, 4:5], AP(w_ext["Wu"], 0, [[1, C], [C, 1]]))
            bhead = cpool.tile([5, 1], f32)
            nc.sync.dma_start(bhead[0:3, :], AP(w_ext["bf"], 0, [[1, 3], [1, 1]]))
            nc.sync.dma_start(bhead[3:4, :], AP(w_ext["bd"], 0, [[1, 1], [1, 1]]))
            nc.sync.dma_start(bhead[4:5, :], AP(w_ext["bu"], 0, [[1, 1], [1, 1]]))

            WgateT = cpool.tile([C, C], f32)
            nc.sync.dma_start(WgateT[:], AP(w_ext["Wgate"], 0, [[1, C], [C, C]]))
            Wg1T = cpool.tile([3, C], f32)
            nc.sync.dma_start(Wg1T[:], AP(w_ext["Wg1"], 0, [[1, 3], [3, C]]))
            Wg2T = cpool.tile([C, C], f32)
            nc.sync.dma_start(Wg2T[:], AP(w_ext["Wg2"], 0, [[1, C], [C, C]]))
            Wr1fT = cpool.tile([C, C], f32)
            nc.sync.dma_start(Wr1fT[:], AP(w_ext["Wr1"], 0, [[1, C], [C + 5, C]]))
            Wr1flT = cpool.tile([3, C], f32)
            nc.sync.dma_start(Wr1flT[:], AP(w_ext["Wr1"], C, [[1, 3], [C + 5, C]]))
            Wr1dT = cpool.tile([2, C], f32)
            nc.sync.dma_start(Wr1dT[:], AP(w_ext["Wr1"], C + 3, [[1, 2], [C + 5, C]]))
            Wr2T = cpool.tile([C, C], f32)
            nc.sync.dma_start(Wr2T[:], AP(w_ext["Wr2"], 0, [[1, C], [C, C]]))

            def vec_col(name):
                t = cpool.tile([C, 1], f32, tag=name, name=name + "_v")
                nc.sync.dma_start(t[:], AP(w_ext[name], 0, [[1, C], [1, 1]]))
                return t
            bgate_v = vec_col("bgate")
            bg1_v = vec_col("bg1")
            bg2_v = vec_col("bg2")
            br2_v = vec_col("br2")
            gamma_v = vec_col("gamma")
            beta_v = vec_col("beta")

            zero128 = cpool.tile([128, 1], f32)
            nc.vector.memset(zero128[:], 0.0)
            segb64u = cpool.tile([128, 64], u16)
            nc.gpsimd.iota(segb64u[:], pattern=[[384, 8], [0, 8]],
                           channel_multiplier=0)
            rank16 = cpool.tile([128, 16], i16)
            nc.gpsimd.iota(rank16[:], pattern=[[1, 16]], base=1,
                           channel_multiplier=0)

            dtv = cpool.tile([128, 1], f32)
            nc.sync.dma_start(dtv[:], AP(w_ext["log_dt"], 0, [[0, 128], [1, 1]]))
            nc.scalar.activation(dtv[:], dtv[:], Act.Exp, bias=zero128[:], scale=1.0)
            nc.vector.tensor_scalar(dtv[:], dtv[:], 1e-4, 10.0, Alu.max, Alu.min)

            # ---------------- S0 loads -----------------
            f_sb = big.tile([128, NB, C], f32)
            nc.sync.dma_start(f_sb[:], AP(f_ext, 0, [[C, 128], [128 * C, NB], [1, C]]))
            xyz_sb = big.tile([128, NB, 3], f32)
            nc.sync.dma_start(xyz_sb[:], AP(xyz_ext, 0, [[3, 128], [128 * 3, NB], [1, 3]]))


            # ---------------- S1 transposes + convs -----------------
            fxT = big.tile([128, N], f32)
            fT = fxT[0:C, :]
            for j in range(6):
                pt = ps(C, 512)
                for q in range(4):
                    b = 4 * j + q
                    nc.tensor.matmul(pt[:, 128 * q:128 * (q + 1)],
                                     f_sb[:, b:b + 1, :], ident[:, :],
                                     is_transpose=True)
                nc.scalar.copy(fxT[0:C, 512 * j:512 * (j + 1)], pt[:])

            pa = ctx.enter_pa(tc.tile_pool(name="passa", bufs=2))
            A1_m = pa.tile([4, N], f32, tag="A1_m", bufs=1)   # [2x; 1]
            B1_m = pa.tile([4, N], f32, tag="B1_m", bufs=1)   # [x; -sq]
            nc.vector.memset(A1_m[:], 1.0)    # row 3 keeps +1
            for j in range(6):
                pt = ps(3, 512)
                for q in range(4):
                    b = 4 * j + q
                    nc.tensor.matmul(pt[:, 128 * q:128 * (q + 1)],
                                     xyz_sb[:, b:b + 1, :], ident[:, :],
                                     is_transpose=True)
                nc.scalar.mul(A1_m[0:3, 512 * j:512 * (j + 1)], pt[:], 2.0)
                nc.vector.tensor_copy(B1_m[0:3, 512 * j:512 * (j + 1)], pt[:])
                nc.scalar.copy(fxT[C:C + 3, 512 * j:512 * (j + 1)], pt[:])

            xyz2 = small.tile([128, NB, 3], f32)
            nc.vector.tensor_tensor(xyz2[:], xyz_sb[:], xyz_sb[:], Alu.mult)
            sq_p = small.tile([128, NB, 1], f32)
            nc.vector.tensor_reduce(sq_p[:], xyz2[:], axis=AX.X, op=Alu.add)
            sqn_p = small.tile([128, NB, 1], f32)
            nc.vector.tensor_scalar(sqn_p[:], sq_p[:], -1.0, None, Alu.mult)
            pt = ps(NB, 128)
            nc.tensor.matmul(pt[:], sq_p[:], ident[:, :], is_transpose=True)
            sq24 = small.tile([NB, 128], f32)
            nc.vector.tensor_copy(sq24[:], pt[:])
            pt = ps(NB, 128)
            nc.tensor.matmul(pt[:], sqn_p[:], ident[:, :], is_transpose=True)
            sqn24 = small.tile([NB, 128], f32)
            nc.vector.tensor_copy(sqn24[:], pt[:])
            nc.sync.dma_start(B1_m[3:4, :], sqn24[:])

            headsT = big.tile([5, N], f32)
            gateT = big.tile([C, N], f32)
            for j in range(6):
                sl = slice(512 * j, 512 * (j + 1))
                ph = ps(5, 512)
                nc.tensor.matmul(ph[:], WhT[:], fT[:, sl], start=True, stop=True)
                nc.vector.tensor_scalar(headsT[:, sl], ph[:], bhead[:], None, Alu.add)
                pg = ps(C, 512)
                nc.tensor.matmul(pg[:], WgateT[:], fT[:, sl], start=True, stop=True)
                nc.scalar.activation(gateT[:, sl], pg[:], Act.Sigmoid,
                                     bias=bgate_v[:], scale=1.0)

            hp = small.tile([128, NB, 5], f32)
            pt5 = ps(128, NB * 5)
            for b in range(NB):
                nc.tensor.matmul(pt5[:, 5 * b:5 * (b + 1)],
                                 headsT[:, 128 * b:128 * (b + 1)], ident[0:5, 0:5],
                                 is_transpose=True)
            nc.vector.tensor_copy(hp[:], pt5[:])

            flow_p = hp[:, :, 0:3]
            de = small.tile([128, NB, 1], f32)
            tmp_b = small.tile([128, NB, 1], f32)
            nc.scalar.activation(tmp_b[:], hp[:, :, 3:4], Act.Exp,
                                 bias=zero128[:], scale=1.0)
            nc.vector.tensor_scalar(tmp_b[:], tmp_b[:], 1.0, None, Alu.add)
            nc.scalar.activation(tmp_b[:], tmp_b[:], Act.Ln,
                                 bias=zero128[:], scale=1.0)
            sgu = small.tile([128, NB, 1], f32)
            nc.scalar.activation(sgu[:], hp[:, :, 4:5], Act.Sigmoid,
                                 bias=zero128[:], scale=1.0)
            nc.vector.tensor_scalar(sgu[:], sgu[:], 1.0, None, Alu.add)
            nc.vector.tensor_tensor(de[:], tmp_b[:], sgu[:], Alu.mult)
            de16 = small.tile([128, NB, 1], f32)
            nc.vector.tensor_scalar(de16[:], de[:], 1.0 / K, None, Alu.mult)

            fl2 = small.tile([128, NB, 3], f32)
            nc.vector.tensor_tensor(fl2[:], flow_p, flow_p, Alu.mult)
            vn = small.tile([128, NB, 1], f32)
            nc.vector.tensor_reduce(vn[:], fl2[:], axis=AX.X, op=Alu.add)
            nc.scalar.activation(vn[:], vn[:], Act.Sqrt, bias=zero128[:], scale=1.0)
            nc.vector.tensor_scalar(vn[:], vn[:], 1e-6, None, Alu.max)
            rv = small.tile([128, NB, 1], f32)
            nc.vector.reciprocal(rv[:], vn[:])
            vhat = small.tile([128, NB, 3], f32)
            nc.vector.tensor_tensor(vhat[:], flow_p,
                                    rv[:].broadcast_to((128, NB, 3)), Alu.mult)

            fgm = small.tile([3, 1], f32)
            nc.vector.tensor_reduce(fgm[:], headsT[0:3, :], axis=AX.X, op=Alu.add)
            nc.vector.tensor_scalar(fgm[:], fgm[:], 1.0 / N, None, Alu.mult)
            pg1 = ps(C, 1)
            nc.tensor.matmul(pg1[:], Wg1T[:], fgm[:], start=True, stop=True)
            hg = small.tile([C, 1], f32)
            nc.scalar.activation(hg[:], pg1[:], Act.Relu, bias=bg1_v[:], scale=1.0)
            pg2 = ps(C, 1)
            nc.tensor.matmul(pg2[:], Wg2T[:], hg[:], start=True, stop=True)
            fgf = small.tile([C, 1], f32)
            nc.vector.tensor_scalar(fgf[:], pg2[:], bg2_v[:], None, Alu.add)

            # ---------------- S2 pass A -----------------
            idx_all = big.tile([128, NB * K], u16)
            for b in range(NB):
                negd = pa.tile([128, N], f32, tag="negd")
                for j in range(6):
                    pd = ps(128, 512)
                    nc.tensor.matmul(pd[:], A1_m[:, 128 * b:128 * (b + 1)],
                                     B1_m[:, 512 * j:512 * (j + 1)],
                                     start=True, stop=True)
                    nc.scalar.activation(negd[:, 512 * j:512 * (j + 1)], pd[:],
                                         Act.Identity,
                                         bias=sqn_p[:, b:b + 1, 0:1].rearrange(
                                             "p a b -> p (a b)"),
                                         scale=1.0)
                if debug and b == 0:
                    nc.sync.dma_start(AP(dbg_ext["d_negd"], 0, [[N, 128], [1, N]]),
                                      negd[:])
                cand = small.tile([128, 64], f32, tag="cand", bufs=2)
                segloc = small.tile([128, 64], u16, tag="segloc", bufs=2)
                for s8 in range(8):
                    nc.vector.max(cand[:, 8 * s8:8 * (s8 + 1)],
                                  negd[:, 384 * s8:384 * (s8 + 1)])
                    nc.vector.max_index(segloc[:, 8 * s8:8 * (s8 + 1)],
                                        cand[:, 8 * s8:8 * (s8 + 1)],
                                        negd[:, 384 * s8:384 * (s8 + 1)])
                jc16 = small.tile([128, 64], u16, tag="jc16", bufs=2)
                nc.vector.tensor_tensor(jc16[:], segloc[:], segb64u[:], Alu.add)
                v16 = small.tile([128, 16], f32, tag="v16", bufs=2)
                mrc = small.tile([128, 64], f32, tag="mrc", bufs=2)
                cp16 = small.tile([128, 16], u16, tag="cp16", bufs=2)
                nc.vector.max(v16[:, 0:8], cand[:])
                nc.vector.max_index(cp16[:, 0:8], v16[:, 0:8], cand[:])
                nc.vector.match_replace(mrc[:], v16[:, 0:8], cand[:], -1e30)
                nc.vector.max(v16[:, 8:16], mrc[:])
                nc.vector.max_index(cp16[:, 8:16], v16[:, 8:16], mrc[:])
                rankmap = small.tile([128, 64], i16, tag="rankmap", bufs=2)
                nc.gpsimd.local_scatter(rankmap[:], rank16[:],
                                        cp16[:].bitcast(i16),
                                        channels=128, num_elems=64, num_idxs=16)
                nc.vector.tensor_scalar(rankmap[:], rankmap[:], 1, None,
                                        Alu.subtract)
                nc.gpsimd.local_scatter(idx_all[:, K * b:K * (b + 1)].bitcast(i16),
                                        jc16[:].bitcast(i16), rankmap[:],
                                        channels=128, num_elems=16, num_idxs=64)

            # ---------------- S3 gather prep -----------------
            # Stage idx to DRAM so that each gather call (bgrp, k) reads a
            # contiguous wrapped [16, 32] block:
            #   dram2 addr = ((b//BT)*K + k)*512 + (p%16)*32 + (b%BT)*8 + p//16
            NBG = NB // BT
            idx_dram = dram.tile([NBG * K * 512], i16)
            for ph in range(8):
                for bg in range(NBG):
                    nc.sync.dma_start(
                        AP(idx_dram.tensor, bg * 512 * K + ph,
                           [[32, 16], [8, BT], [512, K]]),
                        idx_all[16 * ph:16 * (ph + 1),
                                bg * BT * K:(bg + 1) * BT * K].bitcast(i16)
                        .rearrange("p (bl k) -> p bl k", k=K))
            idx_wrap = small.tile([128, NBG * K, 32], i16)
            for g in range(8):
                for bg in range(NBG):
                    nc.sync.dma_start(
                        idx_wrap[16 * g:16 * (g + 1), bg * K:(bg + 1) * K, :],
                        AP(idx_dram.tensor, bg * 512 * K,
                           [[32, 16], [512, K], [1, 32]]))

            if debug:
                idxf = small.tile([128, NB * K], f32, tag="idxf")
                nc.vector.tensor_copy(idxf[:], idx_all[:])
                nc.sync.dma_start(AP(dbg_ext["d_idx"], 0, [[NB * K, 128], [1, NB * K]]),
                                  idxf[:])

            # ---------------- S4 pass C -----------------
            agg = big.tile([128, NB, C], f32)
            dp = small.tile([128, NB, 2], f32)
            if debug:
                dnum = big.tile([128, NB, K], f32, tag="dnum")
                duw = big.tile([128, NB, K], f32, tag="duw")
            pc = ctx.enter_pc(tc.tile_pool(name="passc", bufs=2))
            pcw = ctx.enter_pc(tc.tile_pool(name="passcw", bufs=1))
            for s in range(NSL):
                b0 = BT * s
                fnei = pc.tile([128, BT, K, 80], f32, tag="fnei", bufs=1)
                gth = pc.tile([128, K * BT * 128], f32, tag="gth", bufs=1)
                nc.gpsimd.ap_gather(
                    gth[:],
                    fxT[:],
                    idx_wrap[:, s * K:(s + 1) * K, :].rearrange("p a q -> p (a q)"),
                    channels=128, num_elems=N, d=1, num_idxs=K * BT * 128)
                for kq in range(K):
                    ptg = ps(128, BT * 128)
                    for q in range(BT):
                        nc.tensor.matmul(
                            ptg[:, 128 * q:128 * (q + 1)],
                            gth[:, kq * BT * 128 + 128 * q:
                                kq * BT * 128 + 128 * (q + 1)],
                            ident[:, :], is_transpose=True)
                    nc.scalar.copy(fnei[:, :, kq:kq + 1, :],
                                   ptg[:].rearrange("p (b c) -> p b c", c=128)[:, :, 0:80])
                xyz_nei = fnei[:, :, :, C:C + 3]
                f_nei = fnei[:, :, :, 0:C]
                xsl = xyz_sb[:, b0:b0 + BT, :]
                dxyz = pcw.tile([128, BT, K, 3], f32, tag="dxyz")
                nc.vector.tensor_tensor(
                    dxyz[:], xyz_nei,
                    xsl.unsqueeze(2).broadcast_to((128, BT, K, 3)), Alu.subtract)
                t3 = pcw.tile([128, BT, K, 3], f32, tag="t3")
                nc.vector.tensor_tensor(t3[:], dxyz[:], dxyz[:], Alu.mult)
                d2k = pcw.tile([128, BT, K], f32, tag="d2k")
                nc.vector.tensor_reduce(d2k[:], t3[:], axis=AX.X, op=Alu.add)
                sqd = pcw.tile([128, BT, K], f32, tag="sqd")
                nc.scalar.activation(sqd[:], d2k[:], Act.Ln,
                                     bias=zero128[:], scale=1.0)
                nc.scalar.activation(sqd[:], sqd[:], Act.Exp,
                                     bias=zero128[:], scale=0.5)
                den = pcw.tile([128, BT, K], f32, tag="den")
                nc.vector.tensor_scalar(den[:], sqd[:], 1e-6, None, Alu.max)
                rden = pcw.tile([128, BT, K], f32, tag="rden")
                nc.vector.reciprocal(rden[:], den[:])
                nc.vector.tensor_tensor(
                    t3[:], dxyz[:],
                    vhat[:, b0:b0 + BT, :].unsqueeze(2).broadcast_to((128, BT, K, 3)),
                    Alu.mult)
                numv = pcw.tile([128, BT, K], f32, tag="numv")
                nc.vector.tensor_reduce(numv[:], t3[:], axis=AX.X, op=Alu.add)
                if debug:
                    nc.vector.tensor_copy(dnum[:, b0:b0 + BT, :], numv[:])
                ek = pcw.tile([128, BT, K], f32, tag="ek")
                nc.vector.tensor_tensor(ek[:], numv[:], rden[:], Alu.mult)
                nc.scalar.activation(ek[:], ek[:], Act.Exp,
                                     bias=zero128[:], scale=1.0 / TAU)
                se = pcw.tile([128, BT, 1], f32, tag="se")
                nc.vector.tensor_reduce(se[:], ek[:], axis=AX.X, op=Alu.add)
                rse = pcw.tile([128, BT, 1], f32, tag="rse")
                nc.vector.reciprocal(rse[:], se[:])
                uw = pcw.tile([128, BT, K], f32, tag="uw")
                nc.vector.tensor_tensor(uw[:], ek[:],
                                        rse[:].broadcast_to((128, BT, K)), Alu.mult)
                nc.vector.tensor_tensor(
                    uw[:], uw[:],
                    de16[:, b0:b0 + BT, :].broadcast_to((128, BT, K)), Alu.add)
                if debug:
                    nc.vector.tensor_copy(duw[:, b0:b0 + BT, :], uw[:])
                    if s == 0:
                        nc.sync.dma_start(
                            AP(dbg_ext["d_fnei"], 0,
                               [[BT * K * 80, 128], [1, BT * K * 80]]), fnei[:])
                prod = pcw.tile([128, BT, K, C], f32, tag="prod")
                nc.vector.tensor_tensor(
                    prod[:], f_nei,
                    uw[:].unsqueeze(3).broadcast_to((128, BT, K, C)), Alu.mult)
                s1 = pcw.tile([128, BT, 8, C], f32, tag="s1")
                prodv = prod[:].rearrange("p b (k2 two) c -> p b k2 (two c)", two=2)
                nc.vector.tensor_tensor(s1[:], prodv[:, :, :, 0:C],
                                        prodv[:, :, :, C:2 * C], Alu.add)
                s2 = pcw.tile([128, BT, 4, C], f32, tag="s2")
                s1v = s1[:].rearrange("p b (k2 two) c -> p b k2 (two c)", two=2)
                nc.vector.tensor_tensor(s2[:], s1v[:, :, :, 0:C],
                                        s1v[:, :, :, C:2 * C], Alu.add)
                s3 = pcw.tile([128, BT, 2, C], f32, tag="s3")
                s2v = s2[:].rearrange("p b (k2 two) c -> p b k2 (two c)", two=2)
                nc.vector.tensor_tensor(s3[:], s2v[:, :, :, 0:C],
                                        s2v[:, :, :, C:2 * C], Alu.add)
                s3v = s3[:].rearrange("p b (one two) c -> p b one (two c)", two=2)
                nc.vector.tensor_tensor(agg[:, b0:b0 + BT, :], s3v[:, :, :, 0:C],
                                        s3v[:, :, :, C:2 * C], Alu.add)
                # dist stats
                ndsl = dp[:, b0:b0 + BT, 0:1]
                nvsl = dp[:, b0:b0 + BT, 1:2]
                nc.vector.tensor_reduce(ndsl, sqd[:], axis=AX.X, op=Alu.add)
                nc.vector.tensor_scalar(ndsl, ndsl, 1.0 / K, None, Alu.mult)
                d2m = pcw.tile([128, BT, 1], f32, tag="d2m")
                nc.vector.tensor_reduce(d2m[:], d2k[:], axis=AX.X, op=Alu.add)
                nc.vector.tensor_scalar(d2m[:], d2m[:], 1.0 / K, None, Alu.mult)
                nd2 = pcw.tile([128, BT, 1], f32, tag="nd2")
                nc.vector.tensor_tensor(nd2[:], ndsl, ndsl, Alu.mult)
                nc.vector.tensor_tensor(nvsl, d2m[:], nd2[:], Alu.subtract)

            if debug:
                nc.sync.dma_start(AP(dbg_ext["d_agg"], 0, [[NB * C, 128], [1, NB * C]]),
                                  agg[:])
                nc.sync.dma_start(AP(dbg_ext["d_de"], 0, [[NB, 128], [1, NB]]), de[:])
                nc.sync.dma_start(AP(dbg_ext["d_dist"], 0, [[NB * 2, 128], [1, NB * 2]]),
                                  dp[:])
                nc.sync.dma_start(AP(dbg_ext["d_heads"], 0, [[N, 5], [1, N]]),
                                  headsT[:])
                nc.sync.dma_start(AP(dbg_ext["d_num"], 0, [[NB * K, 128], [1, NB * K]]), dnum[:])
                nc.sync.dma_start(AP(dbg_ext["d_uw"], 0, [[NB * K, 128], [1, NB * K]]), duw[:])
                nc.sync.dma_start(AP(dbg_ext["d_vhat"], 0, [[NB * 3, 128], [1, NB * 3]]), vhat[:])

            # ---------------- S5 reaction + BN -----------------
            ctx.close_pc()
            late = ctx.enter_pa(tc.tile_pool(name="late", bufs=1))
            distT = late.tile([2, N], f32)
            for j in range(6):
                ptd = ps(2, 512)
                for q in range(4):
                    b = 4 * j + q
                    nc.tensor.matmul(ptd[:, 128 * q:128 * (q + 1)],
                                     dp[:, b:b + 1, :], ident[:, :],
                                     is_transpose=True)
                nc.vector.tensor_copy(distT[:, 512 * j:512 * (j + 1)], ptd[:])

            x_sb = late.tile([C, N], f32)
            xs6 = small.tile([C, 6], f32)
            x2s6 = small.tile([C, 6], f32)
            scr = late.tile([C, N], f32)
            for j in range(6):
                sl = slice(512 * j, 512 * (j + 1))
                px = ps(C, 512)
                nc.tensor.matmul(px[:], Wr1fT[:], fT[:, sl], start=True, stop=False)
                nc.tensor.matmul(px[:], Wr1flT[:], headsT[0:3, sl],
                                 start=False, stop=False)
                nc.tensor.matmul(px[:], Wr1dT[:], distT[:, sl],
                                 start=False, stop=True)
                nc.scalar.activation(x_sb[:, sl], px[:], Act.Copy, bias=0.0,
                                     scale=1.0, accum_out=xs6[:, j:j + 1])
                nc.scalar.activation(scr[:, sl], x_sb[:, sl], Act.Square,
                                     bias=zero128[0:C, :], scale=1.0,
                                     accum_out=x2s6[:, j:j + 1])
            bn_loc = small.tile([C, 2], f32)
            nc.vector.tensor_reduce(bn_loc[:, 0:1], xs6[:], axis=AX.X, op=Alu.add)
            nc.vector.tensor_reduce(bn_loc[:, 1:2], x2s6[:], axis=AX.X, op=Alu.add)
            bn_in = dram.tile([C, 2], f32)
            bn_out = dram.tile([C, 2], f32)
            bn_g = small.tile([C, 2], f32)
            if nocol:
                nc.vector.tensor_scalar(bn_g[:], bn_loc[:], float(NCORES), None,
                                        Alu.mult)
            else:
                nc.sync.dma_start(bn_in[:], bn_loc[:])
                nc.gpsimd.collective_compute(
                    "AllReduce", Alu.add, replica_groups=[list(range(NCORES))],
                    ins=[bn_in[:].opt()], outs=[bn_out[:].opt()])
                nc.sync.dma_start(bn_g[:], bn_out[:])
            Mtot = float(NCORES * N)
            mu = small.tile([C, 1], f32)
            nc.vector.tensor_scalar(mu[:], bn_g[:, 0:1], 1.0 / Mtot, None, Alu.mult)
            var = small.tile([C, 1], f32)
            nc.vector.tensor_scalar(var[:], bn_g[:, 1:2], 1.0 / Mtot, None, Alu.mult)
            mu2 = small.tile([C, 1], f32)
            nc.vector.tensor_tensor(mu2[:], mu[:], mu[:], Alu.mult)
            nc.vector.tensor_tensor(var[:], var[:], mu2[:], Alu.subtract)
            nc.vector.tensor_scalar(var[:], var[:], BN_EPS, None, Alu.add)
            nc.scalar.activation(var[:], var[:], Act.Sqrt,
                                 bias=zero128[0:C, :], scale=1.0)
            rstd = small.tile([C, 1], f32)
            nc.vector.reciprocal(rstd[:], var[:])
            s_vec = small.tile([C, 1], f32)
            nc.vector.tensor_tensor(s_vec[:], gamma_v[:], rstd[:], Alu.mult)
            b_vec = small.tile([C, 1], f32)
            nc.vector.tensor_tensor(b_vec[:], mu[:], s_vec[:], Alu.mult)
            nc.vector.tensor_tensor(b_vec[:], beta_v[:], b_vec[:], Alu.subtract)
            if debug:
                nc.sync.dma_start(AP(dbg_ext["d_bn"], 0, [[4, C], [1, 2]]), bn_g[:])
                nc.sync.dma_start(AP(dbg_ext["d_bn"], 2, [[4, C], [1, 1]]), mu[:])
                nc.sync.dma_start(AP(dbg_ext["d_bn"], 3, [[4, C], [1, 1]]), var[:])

            nc.scalar.activation(x_sb[:], x_sb[:], Act.Relu,
                                 bias=b_vec[:], scale=s_vec[:])
            nc.vector.tensor_scalar(gateT[:], gateT[:], fgf[:], None, Alu.mult)
            for j in range(6):
                sl = slice(512 * j, 512 * (j + 1))
                pr = ps(C, 512)
                nc.tensor.matmul(pr[:], Wr2T[:], x_sb[:, sl], start=True, stop=True)
                nc.scalar.copy(scr[:, sl], pr[:])
            nc.vector.tensor_scalar(scr[:], scr[:], br2_v[:], None, Alu.add)
            nc.vector.tensor_tensor(gateT[:], gateT[:], scr[:], Alu.add)

            TRp = late.tile([128, NB, C], f32)
            for j in range(3):
                pt = ps(128, 512)
                for q in range(8):
                    b = 8 * j + q
                    nc.tensor.matmul(pt[:, C * q:C * (q + 1)],
                                     gateT[:, 128 * b:128 * (b + 1)],
                                     ident[0:C, 0:C], is_transpose=True)
                nc.scalar.copy(TRp[:, 8 * j:8 * (j + 1), :], pt[:])

            # ---------------- S6 final combine -----------------
            de1 = small.tile([128, NB, 1], f32)
            nc.vector.tensor_scalar(de1[:], de[:], 1.0, None, Alu.add)
            out_sb = late.tile([128, NB, C], f32)
            nc.vector.tensor_tensor(out_sb[:], f_sb[:],
                                    de1[:].broadcast_to((128, NB, C)), Alu.mult)
            nc.vector.tensor_tensor(agg[:], agg[:], out_sb[:], Alu.subtract)
            nc.vector.tensor_tensor(agg[:], agg[:], TRp[:], Alu.add)
            nc.vector.scalar_tensor_tensor(out_sb[:], agg[:], dtv[:], f_sb[:],
                                           Alu.mult, Alu.add)
            nc.sync.dma_start(AP(out_ext, 0, [[C, 128], [128 * C, NB], [1, C]]),
                              out_sb[:])

    nc.compile()
    return nc


@functools.cache
def _get_nc(debug=False):
    return _build(debug=debug)


def _run(nc, inputs, trace=False):
    from concourse.bass_utils import run_bass_kernel_spmd
    f_seq = np.ascontiguousarray(np.asarray(inputs["f_seq"], dtype=np.float32))
    xyz = np.ascontiguousarray(np.asarray(inputs["xyz"], dtype=np.float32))
    in_maps = []
    for core in range(NCORES):
        b, l = divmod(core, L)
        m = {"f": f_seq[b, l], "xyz": xyz[b, l]}
        for k in WEIGHT_NAMES:
            m[k] = np.ascontiguousarray(
                np.asarray(inputs[k], dtype=np.float32).reshape(-1))
        in_maps.append(m)
    return run_bass_kernel_spmd(nc, in_maps, core_ids=list(range(NCORES)),
                                trace=trace)


def kernel(**inputs):
    nc = _get_nc()
    res = _run(nc, inputs)
    out = np.stack([np.asarray(res.results[i]["out"]) for i in range(NCORES)])
    return out.reshape(B, L, N, C).astype(np.float32)


# revision 39
# speedup vs baseline: 1138.2721x; 1.0030x over previous
"""Trainium2 Bass kernel for ADRiverDynamics (gnn_message_passing).

8 independent point clouds (B*L=8), one per NeuronCore (pure data parallel),
plus one tiny AllReduce for global BatchNorm statistics.

Per-core pipeline (cloud of N=3072 points, C=64 channels, K=16 neighbors):
  S0  load f/xyz, weights; build combined DRAM rows [f|xyz|pad] for gathers
  S1  PE transposes (fT, xyzT->A/B), head convs (flow/diff/unc), gate conv
  S2  pass A: negD = -dist^2 via matmul (two accumulating calls that bit-match
      the reference's d2 formula); per-row top-16 of 3072 via 8-way segmented
      max8 + max_index, merged with match_replace, index indirection resolved
      with two gpsimd local_scatter ops (rank trick)
  S3  neighbor f/xyz gather: gpsimd ap_gather of fxT columns (idx staged via a
      DRAM round-trip into the per-core wrapped layout) + PE transposes back
      to point-major layout
  S4  pass C: K-dense math (cos/softmax weights), fused weighted aggregation
  S5  reaction conv + global-batch BN (AllReduce) + relu + conv
  S6  combine: out = f + dt*(adv + diff + reac)
"""
import functools
import numpy as np

B, L, N, C, K = 2, 4, 3072, 64, 16
NB = N // 128          # 24 point blocks
TAU = 0.15
BN_EPS = 1e-5
NCORES = 8
BT = 4                 # blocks per pass-C slice
NSL = NB // BT         # pass-C slices

WEIGHT_NAMES = ["Wf", "bf", "Wd", "bd", "Wu", "bu", "Wg1", "bg1", "Wg2", "bg2",
                "Wgate", "bgate", "Wr1", "br1", "gamma", "beta", "Wr2", "br2",
                "log_dt"]


def _build(debug=False, nocol=False):
    import contextlib
    from concourse import bacc
    import concourse.bass as bass
    import concourse.tile as tile
    import concourse.mybir as mybir
    from concourse import masks

    f32 = mybir.dt.float32
    u16 = mybir.dt.uint16
    i16 = mybir.dt.int16
    Alu = mybir.AluOpType
    Act = mybir.ActivationFunctionType
    AX = mybir.AxisListType
    AP = bass.AP

    nc = bacc.Bacc("TRN2", target_bir_lowering=False, debug=False,
                   num_devices=NCORES)

    f_ext = nc.dram_tensor("f", [N, C], f32, kind="ExternalInput")
    xyz_ext = nc.dram_tensor("xyz", [N, 3], f32, kind="ExternalInput")
    wshapes = {"Wf": [3, C], "bf": [3], "Wd": [1, C], "bd": [1], "Wu": [1, C],
               "bu": [1], "Wg1": [C, 3], "bg1": [C], "Wg2": [C, C], "bg2": [C],
               "Wgate": [C, C], "bgate": [C], "Wr1": [C, C + 5], "br1": [C],
               "gamma": [C], "beta": [C], "Wr2": [C, C], "br2": [C],
               "log_dt": [1]}
    w_ext = {k: nc.dram_tensor(k, shp, f32, kind="ExternalInput")
             for k, shp in wshapes.items()}
    out_ext = nc.dram_tensor("out", [N, C], f32, kind="ExternalOutput")
    dbg_ext = {}
    if debug:
        for k, shp in {"d_idx": [128, NB * K], "d_agg": [128, NB * C],
                       "d_de": [128, NB], "d_dist": [128, NB * 2],
                       "d_bn": [C, 4], "d_heads": [5, N],
                       "d_negd": [128, N], "d_num": [128, NB * K],
                       "d_uw": [128, NB * K], "d_fnei": [128, BT * K * 80],
                       "d_vhat": [128, NB * 3]}.items():
            dbg_ext[k] = nc.dram_tensor(k, shp, f32, kind="ExternalOutput")

    with tile.TileContext(nc) as tc:
        class _Stacks(contextlib.ExitStack):
            def __init__(self):
                super().__init__()
                self._pa = contextlib.ExitStack()
                self._pc = contextlib.ExitStack()
            def enter_pa(self, cm):
                return self._pa.enter_context(cm)
            def enter_pc(self, cm):
                return self._pc.enter_context(cm)
            def close_pa(self):
                self._pa.close()
            def close_pc(self):
                self._pc.close()
            def __exit__(self, *a):
                self._pc.close()
                self._pa.close()
                return super().__exit__(*a)
        ctx = _Stacks()
        with ctx:
            cpool = ctx.enter_context(tc.tile_pool(name="consts", bufs=1))
            big = ctx.enter_context(tc.tile_pool(name="big", bufs=1))
            dram = ctx.enter_context(tc.tile_pool(name="dram", bufs=1, space="DRAM"))
            psum = ctx.enter_context(tc.tile_pool(name="psum", bufs=4, space="PSUM"))
            small = ctx.enter_context(tc.tile_pool(name="small", bufs=1))

            def ps(p, fr):
                return psum.tile([p, fr], f32, tag="ps", name="pst")

            # ---------------- constants / weights -----------------
            ident = cpool.tile([128, 128], f32)
            masks.make_identity(nc, ident[:])

            WhT = cpool.tile([C, 5], f32)
            nc.sync.dma_start(WhT[:, 0:3], AP(w_ext["Wf"], 0, [[1, C], [C, 3]]))
            nc.sync.dma_start(WhT[:, 3:4], AP(w_ext["Wd"], 0, [[1, C], [C, 1]]))
            nc.sync.dma_start(WhT[:, 4:5], AP(w_ext["Wu"], 0, [[1, C], [C, 1]]))
            bhead = cpool.tile([5, 1], f32)
            nc.sync.dma_start(bhead[0:3, :], AP(w_ext["bf"], 0, [[1, 3], [1, 1]]))
            nc.sync.dma_start(bhead[3:4, :], AP(w_ext["bd"], 0, [[1, 1], [1, 1]]))
            nc.sync.dma_start(bhead[4:5, :], AP(w_ext["bu"], 0, [[1, 1], [1, 1]]))

            WgateT = cpool.tile([C, C], f32)
            nc.sync.dma_start(WgateT[:], AP(w_ext["Wgate"], 0, [[1, C], [C, C]]))
            Wg1T = cpool.tile([3, C], f32)
            nc.sync.dma_start(Wg1T[:], AP(w_ext["Wg1"], 0, [[1, 3], [3, C]]))
            Wg2T = cpool.tile([C, C], f32)
            nc.sync.dma_start(Wg2T[:], AP(w_ext["Wg2"], 0, [[1, C], [C, C]]))
            Wr1fT = cpool.tile([C, C], f32)
            nc.sync.dma_start(Wr1fT[:], AP(w_ext["Wr1"], 0, [[1, C], [C + 5, C]]))
            Wr1flT = cpool.tile([3, C], f32)
            nc.sync.dma_start(Wr1flT[:], AP(w_ext["Wr1"], C, [[1, 3], [C + 5, C]]))
            Wr1dT = cpool.tile([2, C], f32)
            nc.sync.dma_start(Wr1dT[:], AP(w_ext["Wr1"], C + 3, [[1, 2], [C + 5, C]]))
            Wr2T = cpool.tile([C, C], f32)
            nc.sync.dma_start(Wr2T[:], AP(w_ext["Wr2"], 0, [[1, C], [C, C]]))

            def vec_col(name):
                t = cpool.tile([C, 1], f32, tag=name, name=name + "_v")
                nc.sync.dma_start(t[:], AP(w_ext[name], 0, [[1, C], [1, 1]]))
                return t
            bgate_v = vec_col("bgate")
            bg1_v = vec_col("bg1")
            bg2_v = vec_col("bg2")
            br2_v = vec_col("br2")
            gamma_v = vec_col("gamma")
            beta_v = vec_col("beta")

            zero128 = cpool.tile([128, 1], f32)
            nc.vector.memset(zero128[:], 0.0)
            segb64u = cpool.tile([128, 64], u16)
            nc.gpsimd.iota(segb64u[:], pattern=[[384, 8], [0, 8]],
                           channel_multiplier=0)
            rank16 = cpool.tile([128, 16], i16)
            nc.gpsimd.iota(rank16[:], pattern=[[1, 16]], base=1,
                           channel_multiplier=0)

            dtv = cpool.tile([128, 1], f32)
            nc.sync.dma_start(dtv[:], AP(w_ext["log_dt"], 0, [[0, 128], [1, 1]]))
            nc.scalar.activation(dtv[:], dtv[:], Act.Exp, bias=zero128[:], scale=1.0)
            nc.vector.tensor_scalar(dtv[:], dtv[:], 1e-4, 10.0, Alu.max, Alu.min)

            # ---------------- S0 loads -----------------
            f_sb = big.tile([128, NB, C], f32)
            nc.sync.dma_start(f_sb[:], AP(f_ext, 0, [[C, 128], [128 * C, NB], [1, C]]))
            xyz_sb = big.tile([128, NB, 3], f32)
            nc.sync.dma_start(xyz_sb[:], AP(xyz_ext, 0, [[3, 128], [128 * 3, NB], [1, 3]]))


            # ---------------- S1 transposes + convs -----------------
            fxT = big.tile([128, N], f32)
            fT = fxT[0:C, :]
            for j in range(6):
                pt = ps(C, 512)
                for q in range(4):
                    b = 4 * j + q
                    nc.tensor.matmul(pt[:, 128 * q:128 * (q + 1)],
                                     f_sb[:, b:b + 1, :], ident[:, :],
                                     is_transpose=True)
                nc.scalar.copy(fxT[0:C, 512 * j:512 * (j + 1)], pt[:])

            pa = ctx.enter_pa(tc.tile_pool(name="passa", bufs=2))
            A1_m = pa.tile([4, N], f32, tag="A1_m", bufs=1)   # [2x; 1]
            B1_m = pa.tile([4, N], f32, tag="B1_m", bufs=1)   # [x; -sq]
            nc.vector.memset(A1_m[:], 1.0)    # row 3 keeps +1
            for j in range(6):
                pt = ps(3, 512)
                for q in range(4):
                    b = 4 * j + q
                    nc.tensor.matmul(pt[:, 128 * q:128 * (q + 1)],
                                     xyz_sb[:, b:b + 1, :], ident[:, :],
                                     is_transpose=True)
                nc.scalar.mul(A1_m[0:3, 512 * j:512 * (j + 1)], pt[:], 2.0)
                nc.vector.tensor_copy(B1_m[0:3, 512 * j:512 * (j + 1)], pt[:])
                nc.scalar.copy(fxT[C:C + 3, 512 * j:512 * (j + 1)], pt[:])

            xyz2 = small.tile([128, NB, 3], f32)
            nc.vector.tensor_tensor(xyz2[:], xyz_sb[:], xyz_sb[:], Alu.mult)
            sq_p = small.tile([128, NB, 1], f32)
            nc.vector.tensor_reduce(sq_p[:], xyz2[:], axis=AX.X, op=Alu.add)
            sqn_p = small.tile([128, NB, 1], f32)
            nc.vector.tensor_scalar(sqn_p[:], sq_p[:], -1.0, None, Alu.mult)
            pt = ps(NB, 128)
            nc.tensor.matmul(pt[:], sq_p[:], ident[:, :], is_transpose=True)
            sq24 = small.tile([NB, 128], f32)
            nc.vector.tensor_copy(sq24[:], pt[:])
            pt = ps(NB, 128)
            nc.tensor.matmul(pt[:], sqn_p[:], ident[:, :], is_transpose=True)
            sqn24 = small.tile([NB, 128], f32)
            nc.vector.tensor_copy(sqn24[:], pt[:])
            nc.sync.dma_start(B1_m[3:4, :], sqn24[:])

            headsT = big.tile([5, N], f32)
            gateT = big.tile([C, N], f32)
            for j in range(6):
                sl = slice(512 * j, 512 * (j + 1))
                ph = ps(5, 512)
                nc.tensor.matmul(ph[:], WhT[:], fT[:, sl], start=True, stop=True)
                nc.vector.tensor_scalar(headsT[:, sl], ph[:], bhead[:], None, Alu.add)
                pg = ps(C, 512)
                nc.tensor.matmul(pg[:], WgateT[:], fT[:, sl], start=True, stop=True)
                nc.scalar.activation(gateT[:, sl], pg[:], Act.Sigmoid,
                                     bias=bgate_v[:], scale=1.0)

            hp = small.tile([128, NB, 5], f32)
            pt5 = ps(128, NB * 5)
            for b in range(NB):
                nc.tensor.matmul(pt5[:, 5 * b:5 * (b + 1)],
                                 headsT[:, 128 * b:128 * (b + 1)], ident[0:5, 0:5],
                                 is_transpose=True)
            nc.vector.tensor_copy(hp[:], pt5[:])

            flow_p = hp[:, :, 0:3]
            de = small.tile([128, NB, 1], f32)
            tmp_b = small.tile([128, NB, 1], f32)
            nc.scalar.activation(tmp_b[:], hp[:, :, 3:4], Act.Exp,
                                 bias=zero128[:], scale=1.0)
            nc.vector.tensor_scalar(tmp_b[:], tmp_b[:], 1.0, None, Alu.add)
            nc.scalar.activation(tmp_b[:], tmp_b[:], Act.Ln,
                                 bias=zero128[:], scale=1.0)
            sgu = small.tile([128, NB, 1], f32)
            nc.scalar.activation(sgu[:], hp[:, :, 4:5], Act.Sigmoid,
                                 bias=zero128[:], scale=1.0)
            nc.vector.tensor_scalar(sgu[:], sgu[:], 1.0, None, Alu.add)
            nc.vector.tensor_tensor(de[:], tmp_b[:], sgu[:], Alu.mult)
            de16 = small.tile([128, NB, 1], f32)
            nc.vector.tensor_scalar(de16[:], de[:], 1.0 / K, None, Alu.mult)

            fl2 = small.tile([128, NB, 3], f32)
            nc.vector.tensor_tensor(fl2[:], flow_p, flow_p, Alu.mult)
            vn = small.tile([128, NB, 1], f32)
            nc.vector.tensor_reduce(vn[:], fl2[:], axis=AX.X, op=Alu.add)
            nc.scalar.activation(vn[:], vn[:], Act.Sqrt, bias=zero128[:], scale=1.0)
            nc.vector.tensor_scalar(vn[:], vn[:], 1e-6, None, Alu.max)
            rv = small.tile([128, NB, 1], f32)
            nc.vector.reciprocal(rv[:], vn[:])
            vhat = small.tile([128, NB, 3], f32)
            nc.vector.tensor_tensor(vhat[:], flow_p,
                                    rv[:].broadcast_to((128, NB, 3)), Alu.mult)

            fgm = small.tile([3, 1], f32)
            nc.vector.tensor_reduce(fgm[:], headsT[0:3, :], axis=AX.X, op=Alu.add)
            nc.vector.tensor_scalar(fgm[:], fgm[:], 1.0 / N, None, Alu.mult)
            pg1 = ps(C, 1)
            nc.tensor.matmul(pg1[:], Wg1T[:], fgm[:], start=True, stop=True)
            hg = small.tile([C, 1], f32)
            nc.scalar.activation(hg[:], pg1[:], Act.Relu, bias=bg1_v[:], scale=1.0)
            pg2 = ps(C, 1)
            nc.tensor.matmul(pg2[:], Wg2T[:], hg[:], start=True, stop=True)
            fgf = small.tile([C, 1], f32)
            nc.vector.tensor_scalar(fgf[:], pg2[:], bg2_v[:], None, Alu.add)

            # ---------------- S2 pass A -----------------
            idx_all = big.tile([128, NB * K], u16)
            for b in range(NB):
                negd = pa.tile([128, N], f32, tag="negd")
                for j in range(6):
                    pd = ps(128, 512)
                    nc.tensor.matmul(pd[:], A1_m[:, 128 * b:128 * (b + 1)],
                                     B1_m[:, 512 * j:512 * (j + 1)],
                                     start=True, stop=True)
                    nc.scalar.activation(negd[:, 512 * j:512 * (j + 1)], pd[:],
                                         Act.Identity,
                                         bias=sqn_p[:, b:b + 1, 0:1].rearrange(
                                             "p a b -> p (a b)"),
                                         scale=1.0)
                if debug and b == 0:
                    nc.sync.dma_start(AP(dbg_ext["d_negd"], 0, [[N, 128], [1, N]]),
                                      negd[:])
                cand = small.tile([128, 64], f32, tag="cand", bufs=2)
                segloc = small.tile([128, 64], u16, tag="segloc", bufs=2)
                for s8 in range(8):
                    nc.vector.max(cand[:, 8 * s8:8 * (s8 + 1)],
                                  negd[:, 384 * s8:384 * (s8 + 1)])
                    nc.vector.max_index(segloc[:, 8 * s8:8 * (s8 + 1)],
                                        cand[:, 8 * s8:8 * (s8 + 1)],
                                        negd[:, 384 * s8:384 * (s8 + 1)])
                jc16 = small.tile([128, 64], u16, tag="jc16", bufs=2)
                nc.vector.tensor_tensor(jc16[:], segloc[:], segb64u[:], Alu.add)
                v16 = small.tile([128, 16], f32, tag="v16", bufs=2)
                mrc = small.tile([128, 64], f32, tag="mrc", bufs=2)
                cp16 = small.tile([128, 16], u16, tag="cp16", bufs=2)
                nc.vector.max(v16[:, 0:8], cand[:])
                nc.vector.max_index(cp16[:, 0:8], v16[:, 0:8], cand[:])
                nc.vector.match_replace(mrc[:], v16[:, 0:8], cand[:], -1e30)
                nc.vector.max(v16[:, 8:16], mrc[:])
                nc.vector.max_index(cp16[:, 8:16], v16[:, 8:16], mrc[:])
                rankmap = small.tile([128, 64], i16, tag="rankmap", bufs=2)
                nc.gpsimd.local_scatter(rankmap[:], rank16[:],
                                        cp16[:].bitcast(i16),
                                        channels=128, num_elems=64, num_idxs=16)
                nc.vector.tensor_scalar(rankmap[:], rankmap[:], 1, None,
                                        Alu.subtract)
                nc.gpsimd.local_scatter(idx_all[:, K * b:K * (b + 1)].bitcast(i16),
                                        jc16[:].bitcast(i16), rankmap[:],
                                        channels=128, num_elems=16, num_idxs=64)

            # ---------------- S3 gather prep -----------------
            # Stage idx to DRAM so that each gather call (bgrp, k) reads a
            # contiguous wrapped [16, 32] block:
            #   dram2 addr = ((b//BT)*K + k)*512 + (p%16)*32 + (b%BT)*8 + p//16
            NBG = NB // BT
            idx_dram = dram.tile([NBG * K * 512], i16)
            for ph in range(8):
                for bg in range(NBG):
                    nc.sync.dma_start(
                        AP(idx_dram.tensor, bg * 512 * K + ph,
                           [[32, 16], [8, BT], [512, K]]),
                        idx_all[16 * ph:16 * (ph + 1),
                                bg * BT * K:(bg + 1) * BT * K].bitcast(i16)
                        .rearrange("p (bl k) -> p bl k", k=K))
            idx_wrap = small.tile([128, NBG * K, 32], i16)
            for g in range(8):
                for bg in range(NBG):
                    nc.sync.dma_start(
                        idx_wrap[16 * g:16 * (g + 1), bg * K:(bg + 1) * K, :],
                        AP(idx_dram.tensor, bg * 512 * K,
                           [[32, 16], [512, K], [1, 32]]))

            if debug:
                idxf = small.tile([128, NB * K], f32, tag="idxf")
                nc.vector.tensor_copy(idxf[:], idx_all[:])
                nc.sync.dma_start(AP(dbg_ext["d_idx"], 0, [[NB * K, 128], [1, NB * K]]),
                                  idxf[:])

            # ---------------- S4 pass C -----------------
            agg = big.tile([128, NB, C], f32)
            dp = small.tile([128, NB, 2], f32)
            if debug:
                dnum = big.tile([128, NB, K], f32, tag="dnum")
                duw = big.tile([128, NB, K], f32, tag="duw")
            pc = ctx.enter_pc(tc.tile_pool(name="passc", bufs=2))
            pcw = ctx.enter_pc(tc.tile_pool(name="passcw", bufs=1))
            for s in range(NSL):
                b0 = BT * s
                fnei = pc.tile([128, BT, K, 80], f32, tag="fnei", bufs=1)
                gth = pc.tile([128, K * BT * 128], f32, tag="gth", bufs=1)
                nc.gpsimd.ap_gather(
                    gth[:],
                    fxT[:],
                    idx_wrap[:, s * K:(s + 1) * K, :].rearrange("p a q -> p (a q)"),
                    channels=128, num_elems=N, d=1, num_idxs=K * BT * 128)
                for kq in range(K):
                    ptg = ps(128, BT * 128)
                    for q in range(BT):
                        nc.tensor.matmul(
                            ptg[:, 128 * q:128 * (q + 1)],
                            gth[:, kq * BT * 128 + 128 * q:
                                kq * BT * 128 + 128 * (q + 1)],
                            ident[:, :], is_transpose=True)
                    nc.scalar.copy(fnei[:, :, kq:kq + 1, :],
                                   ptg[:].rearrange("p (b c) -> p b c", c=128)[:, :, 0:80])
                xyz_nei = fnei[:, :, :, C:C + 3]
                f_nei = fnei[:, :, :, 0:C]
                xsl = xyz_sb[:, b0:b0 + BT, :]
                dxyz = pcw.tile([128, BT, K, 3], f32, tag="dxyz", bufs=2)
                nc.vector.tensor_tensor(
                    dxyz[:], xyz_nei,
                    xsl.unsqueeze(2).broadcast_to((128, BT, K, 3)), Alu.subtract)
                t3 = pcw.tile([128, BT, K, 3], f32, tag="t3", bufs=2)
                nc.vector.tensor_tensor(t3[:], dxyz[:], dxyz[:], Alu.mult)
                d2k = pcw.tile([128, BT, K], f32, tag="d2k", bufs=2)
                nc.vector.tensor_reduce(d2k[:], t3[:], axis=AX.X, op=Alu.add)
                sqd = pcw.tile([128, BT, K], f32, tag="sqd", bufs=2)
                nc.scalar.activation(sqd[:], d2k[:], Act.Ln,
                                     bias=zero128[:], scale=1.0)
                nc.scalar.activation(sqd[:], sqd[:], Act.Exp,
                                     bias=zero128[:], scale=0.5)
                den = pcw.tile([128, BT, K], f32, tag="den", bufs=2)
                nc.vector.tensor_scalar(den[:], sqd[:], 1e-6, None, Alu.max)
                rden = pcw.tile([128, BT, K], f32, tag="rden", bufs=2)
                nc.vector.reciprocal(rden[:], den[:])
                nc.vector.tensor_tensor(
                    t3[:], dxyz[:],
                    vhat[:, b0:b0 + BT, :].unsqueeze(2).broadcast_to((128, BT, K, 3)),
                    Alu.mult)
                numv = pcw.tile([128, BT, K], f32, tag="numv", bufs=2)
                nc.vector.tensor_reduce(numv[:], t3[:], axis=AX.X, op=Alu.add)
                if debug:
                    nc.vector.tensor_copy(dnum[:, b0:b0 + BT, :], numv[:])
                ek = pcw.tile([128, BT, K], f32, tag="ek", bufs=2)
                nc.vector.tensor_tensor(ek[:], numv[:], rden[:], Alu.mult)
                nc.scalar.activation(ek[:], ek[:], Act.Exp,
                                     bias=zero128[:], scale=1.0 / TAU)
                se = pcw.tile([128, BT, 1], f32, tag="se", bufs=2)
                nc.vector.tensor_reduce(se[:], ek[:], axis=AX.X, op=Alu.add)
                rse = pcw.tile([128, BT, 1], f32, tag="rse", bufs=2)
                nc.vector.reciprocal(rse[:], se[:])
                uw = pcw.tile([128, BT, K], f32, tag="uw", bufs=2)
                nc.vector.tensor_tensor(uw[:], ek[:],
                                        rse[:].broadcast_to((128, BT, K)), Alu.mult)
                nc.vector.tensor_tensor(
                    uw[:], uw[:],
                    de16[:, b0:b0 + BT, :].broadcast_to((128, BT, K)), Alu.add)
                if debug:
                    nc.vector.tensor_copy(duw[:, b0:b0 + BT, :], uw[:])
                    if s == 0:
                        nc.sync.dma_start(
                            AP(dbg_ext["d_fnei"], 0,
                               [[BT * K * 80, 128], [1, BT * K * 80]]), fnei[:])
                prod = pcw.tile([128, BT, K, C], f32, tag="prod")
                nc.vector.tensor_tensor(
                    prod[:], f_nei,
                    uw[:].unsqueeze(3).broadcast_to((128, BT, K, C)), Alu.mult)
                s1 = pcw.tile([128, BT, 8, C], f32, tag="s1")
                prodv = prod[:].rearrange("p b (k2 two) c -> p b k2 (two c)", two=2)
                nc.vector.tensor_tensor(s1[:], prodv[:, :, :, 0:C],
                                        prodv[:, :, :, C:2 * C], Alu.add)
                s2 = pcw.tile([128, BT, 4, C], f32, tag="s2")
                s1v = s1[:].rearrange("p b (k2 two) c -> p b k2 (two c)", two=2)
                nc.vector.tensor_tensor(s2[:], s1v[:, :, :, 0:C],
                                        s1v[:, :, :, C:2 * C], Alu.add)
                s3 = pcw.tile([128, BT, 2, C], f32, tag="s3")
                s2v = s2[:].rearrange("p b (k2 two) c -> p b k2 (two c)", two=2)
                nc.vector.tensor_tensor(s3[:], s2v[:, :, :, 0:C],
                                        s2v[:, :, :, C:2 * C], Alu.add)
                s3v = s3[:].rearrange("p b (one two) c -> p b one (two c)", two=2)
                nc.vector.tensor_tensor(agg[:, b0:b0 + BT, :], s3v[:, :, :, 0:C],
                                        s3v[:, :, :, C:2 * C], Alu.add)
                # dist stats
                ndsl = dp[:, b0:b0 + BT, 0:1]
                nvsl = dp[:, b0:b0 + BT, 1:2]
                nc.vector.tensor_reduce(ndsl, sqd[:], axis=AX.X, op=Alu.add)
                nc.vector.tensor_scalar(ndsl, ndsl, 1.0 / K, None, Alu.mult)
                d2m = pcw.tile([128, BT, 1], f32, tag="d2m", bufs=2)
                nc.vector.tensor_reduce(d2m[:], d2k[:], axis=AX.X, op=Alu.add)
                nc.vector.tensor_scalar(d2m[:], d2m[:], 1.0 / K, None, Alu.mult)
                nd2 = pcw.tile([128, BT, 1], f32, tag="nd2", bufs=2)
                nc.vector.tensor_tensor(nd2[:], ndsl, ndsl, Alu.mult)
                nc.vector.tensor_tensor(nvsl, d2m[:], nd2[:], Alu.subtract)

            if debug:
                nc.sync.dma_start(AP(dbg_ext["d_agg"], 0, [[NB * C, 128], [1, NB * C]]),
                                  agg[:])
                nc.sync.dma_start(AP(dbg_ext["d_de"], 0, [[NB, 128], [1, NB]]), de[:])
                nc.sync.dma_start(AP(dbg_ext["d_dist"], 0, [[NB * 2, 128], [1, NB * 2]]),
                                  dp[:])
                nc.sync.dma_start(AP(dbg_ext["d_heads"], 0, [[N, 5], [1, N]]),
                                  headsT[:])
                nc.sync.dma_start(AP(dbg_ext["d_num"], 0, [[NB * K, 128], [1, NB * K]]), dnum[:])
                nc.sync.dma_start(AP(dbg_ext["d_uw"], 0, [[NB * K, 128], [1, NB * K]]), duw[:])
                nc.sync.dma_start(AP(dbg_ext["d_vhat"], 0, [[NB * 3, 128], [1, NB * 3]]), vhat[:])

            # ---------------- S5 reaction + BN -----------------
            ctx.close_pc()
            late = ctx.enter_pa(tc.tile_pool(name="late", bufs=1))
            distT = late.tile([2, N], f32)
            for j in range(6):
                ptd = ps(2, 512)
                for q in range(4):
                    b = 4 * j + q
                    nc.tensor.matmul(ptd[:, 128 * q:128 * (q + 1)],
                                     dp[:, b:b + 1, :], ident[:, :],
                                     is_transpose=True)
                nc.vector.tensor_copy(distT[:, 512 * j:512 * (j + 1)], ptd[:])

            x_sb = late.tile([C, N], f32)
            xs6 = small.tile([C, 6], f32)
            x2s6 = small.tile([C, 6], f32)
            scr = late.tile([C, N], f32)
            for j in range(6):
                sl = slice(512 * j, 512 * (j + 1))
                px = ps(C, 512)
                nc.tensor.matmul(px[:], Wr1fT[:], fT[:, sl], start=True, stop=False)
                nc.tensor.matmul(px[:], Wr1flT[:], headsT[0:3, sl],
                                 start=False, stop=False)
                nc.tensor.matmul(px[:], Wr1dT[:], distT[:, sl],
                                 start=False, stop=True)
                nc.scalar.activation(x_sb[:, sl], px[:], Act.Copy, bias=0.0,
                                     scale=1.0, accum_out=xs6[:, j:j + 1])
                nc.scalar.activation(scr[:, sl], x_sb[:, sl], Act.Square,
                                     bias=zero128[0:C, :], scale=1.0,
                                     accum_out=x2s6[:, j:j + 1])
            bn_loc = small.tile([C, 2], f32)
            nc.vector.tensor_reduce(bn_loc[:, 0:1], xs6[:], axis=AX.X, op=Alu.add)
            nc.vector.tensor_reduce(bn_loc[:, 1:2], x2s6[:], axis=AX.X, op=Alu.add)
            bn_in = dram.tile([C, 2], f32)
            bn_out = dram.tile([C, 2], f32)
            bn_g = small.tile([C, 2], f32)
            if nocol:
                nc.vector.tensor_scalar(bn_g[:], bn_loc[:], float(NCORES), None,
                                        Alu.mult)
            else:
                nc.sync.dma_start(bn_in[:], bn_loc[:])
                nc.gpsimd.collective_compute(
                    "AllReduce", Alu.add, replica_groups=[list(range(NCORES))],
                    ins=[bn_in[:].opt()], outs=[bn_out[:].opt()])
                nc.sync.dma_start(bn_g[:], bn_out[:])
            Mtot = float(NCORES * N)
            mu = small.tile([C, 1], f32)
            nc.vector.tensor_scalar(mu[:], bn_g[:, 0:1], 1.0 / Mtot, None, Alu.mult)
            var = small.tile([C, 1], f32)
            nc.vector.tensor_scalar(var[:], bn_g[:, 1:2], 1.0 / Mtot, None, Alu.mult)
            mu2 = small.tile([C, 1], f32)
            nc.vector.tensor_tensor(mu2[:], mu[:], mu[:], Alu.mult)
            nc.vector.tensor_tensor(var[:], var[:], mu2[:], Alu.subtract)
            nc.vector.tensor_scalar(var[:], var[:], BN_EPS, None, Alu.add)
            nc.scalar.activation(var[:], var[:], Act.Sqrt,
                                 bias=zero128[0:C, :], scale=1.0)
            rstd = small.tile([C, 1], f32)
            nc.vector.reciprocal(rstd[:], var[:])
            s_vec = small.tile([C, 1], f32)
            nc.vector.tensor_tensor(s_vec[:], gamma_v[:], rstd[:], Alu.mult)
            b_vec = small.tile([C, 1], f32)
            nc.vector.tensor_tensor(b_vec[:], mu[:], s_vec[:], Alu.mult)
            nc.vector.tensor_tensor(b_vec[:], beta_v[:], b_vec[:], Alu.subtract)
            if debug:
                nc.sync.dma_start(AP(dbg_ext["d_bn"], 0, [[4, C], [1, 2]]), bn_g[:])
                nc.sync.dma_start(AP(dbg_ext["d_bn"], 2, [[4, C], [1, 1]]), mu[:])
                nc.sync.dma_start(AP(dbg_ext["d_bn"], 3, [[4, C], [1, 1]]), var[:])

            nc.scalar.activation(x_sb[:], x_sb[:], Act.Relu,
                                 bias=b_vec[:], scale=s_vec[:])
            nc.vector.tensor_scalar(gateT[:], gateT[:], fgf[:], None, Alu.mult)
            for j in range(6):
                sl = slice(512 * j, 512 * (j + 1))
                pr = ps(C, 512)
                nc.tensor.matmul(pr[:], Wr2T[:], x_sb[:, sl], start=True, stop=True)
                nc.scalar.copy(scr[:, sl], pr[:])
            nc.vector.tensor_scalar(scr[:], scr[:], br2_v[:], None, Alu.add)
            nc.vector.tensor_tensor(gateT[:], gateT[:], scr[:], Alu.add)

            TRp = late.tile([128, NB, C], f32)
            for j in range(3):
                pt = ps(128, 512)
                for q in range(8):
                    b = 8 * j + q
                    nc.tensor.matmul(pt[:, C * q:C * (q + 1)],
                                     gateT[:, 128 * b:128 * (b + 1)],
                                     ident[0:C, 0:C], is_transpose=True)
                nc.scalar.copy(TRp[:, 8 * j:8 * (j + 1), :], pt[:])

            # ---------------- S6 final combine -----------------
            de1 = small.tile([128, NB, 1], f32)
            nc.vector.tensor_scalar(de1[:], de[:], 1.0, None, Alu.add)
            out_sb = late.tile([128, NB, C], f32)
            nc.vector.tensor_tensor(out_sb[:], f_sb[:],
                                    de1[:].broadcast_to((128, NB, C)), Alu.mult)
            nc.vector.tensor_tensor(agg[:], agg[:], out_sb[:], Alu.subtract)
            nc.vector.tensor_tensor(agg[:], agg[:], TRp[:], Alu.add)
            nc.vector.scalar_tensor_tensor(out_sb[:], agg[:], dtv[:], f_sb[:],
                                           Alu.mult, Alu.add)
            nc.sync.dma_start(AP(out_ext, 0, [[C, 128], [128 * C, NB], [1, C]]),
                              out_sb[:])

    nc.compile()
    return nc


@functools.cache
def _get_nc(debug=False):
    return _build(debug=debug)


def _run(nc, inputs, trace=False):
    from concourse.bass_utils import run_bass_kernel_spmd
    f_seq = np.ascontiguousarray(np.asarray(inputs["f_seq"], dtype=np.float32))
    xyz = np.ascontiguousarray(np.asarray(inputs["xyz"], dtype=np.float32))
    in_maps = []
    for core in range(NCORES):
        b, l = divmod(core, L)
        m = {"f": f_seq[b, l], "xyz": xyz[b, l]}
        for k in WEIGHT_NAMES:
            m[k] = np.ascontiguousarray(
                np.asarray(inputs[k], dtype=np.float32).reshape(-1))
        in_maps.append(m)
    return run_bass_kernel_spmd(nc, in_maps, core_ids=list(range(NCORES)),
                                trace=trace)


def kernel(**inputs):
    nc = _get_nc()
    res = _run(nc, inputs)
    out = np.stack([np.asarray(res.results[i]["out"]) for i in range(NCORES)])
    return out.reshape(B, L, N, C).astype(np.float32)


# revision 40
# speedup vs baseline: 1160.2536x; 1.0193x over previous
"""Trainium2 Bass kernel for ADRiverDynamics (gnn_message_passing).

8 independent point clouds (B*L=8), one per NeuronCore (pure data parallel),
plus one tiny AllReduce for global BatchNorm statistics.

Per-core pipeline (cloud of N=3072 points, C=64 channels, K=16 neighbors):
  S0  load f/xyz, weights; build combined DRAM rows [f|xyz|pad] for gathers
  S1  PE transposes (fT, xyzT->A/B), head convs (flow/diff/unc), gate conv
  S2  pass A: negD = -dist^2 via matmul (two accumulating calls that bit-match
      the reference's d2 formula); per-row top-16 of 3072 via 8-way segmented
      max8 + max_index, merged with match_replace, index indirection resolved
      with two gpsimd local_scatter ops (rank trick)
  S3  neighbor f/xyz gather: gpsimd ap_gather of fxT columns (idx staged via a
      DRAM round-trip into the per-core wrapped layout) + PE transposes back
      to point-major layout
  S4  pass C: K-dense math (cos/softmax weights), fused weighted aggregation
  S5  reaction conv + global-batch BN (AllReduce) + relu + conv
  S6  combine: out = f + dt*(adv + diff + reac)
"""
import functools
import numpy as np

B, L, N, C, K = 2, 4, 3072, 64, 16
NB = N // 128          # 24 point blocks
TAU = 0.15
BN_EPS = 1e-5
NCORES = 8
BT = 4                 # blocks per pass-C slice
NSL = NB // BT         # pass-C slices

WEIGHT_NAMES = ["Wf", "bf", "Wd", "bd", "Wu", "bu", "Wg1", "bg1", "Wg2", "bg2",
                "Wgate", "bgate", "Wr1", "br1", "gamma", "beta", "Wr2", "br2",
                "log_dt"]


def _build(debug=False, nocol=False):
    import contextlib
    from concourse import bacc
    import concourse.bass as bass
    import concourse.tile as tile
    import concourse.mybir as mybir
    from concourse import masks

    f32 = mybir.dt.float32
    u16 = mybir.dt.uint16
    i16 = mybir.dt.int16
    Alu = mybir.AluOpType
    Act = mybir.ActivationFunctionType
    AX = mybir.AxisListType
    AP = bass.AP

    nc = bacc.Bacc("TRN2", target_bir_lowering=False, debug=False,
                   num_devices=NCORES)

    f_ext = nc.dram_tensor("f", [N, C], f32, kind="ExternalInput")
    xyz_ext = nc.dram_tensor("xyz", [N, 3], f32, kind="ExternalInput")
    wshapes = {"Wf": [3, C], "bf": [3], "Wd": [1, C], "bd": [1], "Wu": [1, C],
               "bu": [1], "Wg1": [C, 3], "bg1": [C], "Wg2": [C, C], "bg2": [C],
               "Wgate": [C, C], "bgate": [C], "Wr1": [C, C + 5], "br1": [C],
               "gamma": [C], "beta": [C], "Wr2": [C, C], "br2": [C],
               "log_dt": [1]}
    w_ext = {k: nc.dram_tensor(k, shp, f32, kind="ExternalInput")
             for k, shp in wshapes.items()}
    out_ext = nc.dram_tensor("out", [N, C], f32, kind="ExternalOutput")
    dbg_ext = {}
    if debug:
        for k, shp in {"d_idx": [128, NB * K], "d_agg": [128, NB * C],
                       "d_de": [128, NB], "d_dist": [128, NB * 2],
                       "d_bn": [C, 4], "d_heads": [5, N],
                       "d_negd": [128, N], "d_num": [128, NB * K],
                       "d_uw": [128, NB * K], "d_fnei": [128, BT * K * 80],
                       "d_vhat": [128, NB * 3]}.items():
            dbg_ext[k] = nc.dram_tensor(k, shp, f32, kind="ExternalOutput")

    with tile.TileContext(nc) as tc:
        class _Stacks(contextlib.ExitStack):
            def __init__(self):
                super().__init__()
                self._pa = contextlib.ExitStack()
                self._pc = contextlib.ExitStack()
            def enter_pa(self, cm):
                return self._pa.enter_context(cm)
            def enter_pc(self, cm):
                return self._pc.enter_context(cm)
            def close_pa(self):
                self._pa.close()
            def close_pc(self):
                self._pc.close()
            def __exit__(self, *a):
                self._pc.close()
                self._pa.close()
                return super().__exit__(*a)
        ctx = _Stacks()
        with ctx:
            cpool = ctx.enter_context(tc.tile_pool(name="consts", bufs=1))
            big = ctx.enter_context(tc.tile_pool(name="big", bufs=1))
            dram = ctx.enter_context(tc.tile_pool(name="dram", bufs=1, space="DRAM"))
            psum = ctx.enter_context(tc.tile_pool(name="psum", bufs=4, space="PSUM"))
            small = ctx.enter_context(tc.tile_pool(name="small", bufs=1))

            def ps(p, fr):
                return psum.tile([p, fr], f32, tag="ps", name="pst")

            # ---------------- constants / weights -----------------
            ident = cpool.tile([128, 128], f32)
            masks.make_identity(nc, ident[:])

            WhT = cpool.tile([C, 5], f32)
            nc.sync.dma_start(WhT[:, 0:3], AP(w_ext["Wf"], 0, [[1, C], [C, 3]]))
            nc.sync.dma_start(WhT[:, 3:4], AP(w_ext["Wd"], 0, [[1, C], [C, 1]]))
            nc.sync.dma_start(WhT[:, 4:5], AP(w_ext["Wu"], 0, [[1, C], [C, 1]]))
            bhead = cpool.tile([5, 1], f32)
            nc.sync.dma_start(bhead[0:3, :], AP(w_ext["bf"], 0, [[1, 3], [1, 1]]))
            nc.sync.dma_start(bhead[3:4, :], AP(w_ext["bd"], 0, [[1, 1], [1, 1]]))
            nc.sync.dma_start(bhead[4:5, :], AP(w_ext["bu"], 0, [[1, 1], [1, 1]]))

            WgateT = cpool.tile([C, C], f32)
            nc.sync.dma_start(WgateT[:], AP(w_ext["Wgate"], 0, [[1, C], [C, C]]))
            Wg1T = cpool.tile([3, C], f32)
            nc.sync.dma_start(Wg1T[:], AP(w_ext["Wg1"], 0, [[1, 3], [3, C]]))
            Wg2T = cpool.tile([C, C], f32)
            nc.sync.dma_start(Wg2T[:], AP(w_ext["Wg2"], 0, [[1, C], [C, C]]))
            Wr1fT = cpool.tile([C, C], f32)
            nc.sync.dma_start(Wr1fT[:], AP(w_ext["Wr1"], 0, [[1, C], [C + 5, C]]))
            Wr1flT = cpool.tile([3, C], f32)
            nc.sync.dma_start(Wr1flT[:], AP(w_ext["Wr1"], C, [[1, 3], [C + 5, C]]))
            Wr1dT = cpool.tile([2, C], f32)
            nc.sync.dma_start(Wr1dT[:], AP(w_ext["Wr1"], C + 3, [[1, 2], [C + 5, C]]))
            Wr2T = cpool.tile([C, C], f32)
            nc.sync.dma_start(Wr2T[:], AP(w_ext["Wr2"], 0, [[1, C], [C, C]]))

            def vec_col(name):
                t = cpool.tile([C, 1], f32, tag=name, name=name + "_v")
                nc.sync.dma_start(t[:], AP(w_ext[name], 0, [[1, C], [1, 1]]))
                return t
            bgate_v = vec_col("bgate")
            bg1_v = vec_col("bg1")
            bg2_v = vec_col("bg2")
            br2_v = vec_col("br2")
            gamma_v = vec_col("gamma")
            beta_v = vec_col("beta")

            zero128 = cpool.tile([128, 1], f32)
            nc.vector.memset(zero128[:], 0.0)
            segb64u = cpool.tile([128, 64], u16)
            nc.gpsimd.iota(segb64u[:], pattern=[[384, 8], [0, 8]],
                           channel_multiplier=0)
            rank16 = cpool.tile([128, 16], i16)
            nc.gpsimd.iota(rank16[:], pattern=[[1, 16]], base=1,
                           channel_multiplier=0)

            dtv = cpool.tile([128, 1], f32)
            nc.sync.dma_start(dtv[:], AP(w_ext["log_dt"], 0, [[0, 128], [1, 1]]))
            nc.scalar.activation(dtv[:], dtv[:], Act.Exp, bias=zero128[:], scale=1.0)
            nc.vector.tensor_scalar(dtv[:], dtv[:], 1e-4, 10.0, Alu.max, Alu.min)

            # ---------------- S0 loads -----------------
            f_sb = big.tile([128, NB, C], f32)
            nc.sync.dma_start(f_sb[:], AP(f_ext, 0, [[C, 128], [128 * C, NB], [1, C]]))
            xyz_sb = big.tile([128, NB, 3], f32)
            nc.sync.dma_start(xyz_sb[:], AP(xyz_ext, 0, [[3, 128], [128 * 3, NB], [1, 3]]))


            # ---------------- S1 transposes + convs -----------------
            fxT = big.tile([128, N], f32)
            fT = fxT[0:C, :]
            for j in range(6):
                pt = ps(C, 512)
                for q in range(4):
                    b = 4 * j + q
                    nc.tensor.matmul(pt[:, 128 * q:128 * (q + 1)],
                                     f_sb[:, b:b + 1, :], ident[:, :],
                                     is_transpose=True)
                nc.scalar.copy(fxT[0:C, 512 * j:512 * (j + 1)], pt[:])

            pa = ctx.enter_pa(tc.tile_pool(name="passa", bufs=2))
            A1_m = pa.tile([4, N], f32, tag="A1_m", bufs=1)   # [2x; 1]
            B1_m = pa.tile([4, N], f32, tag="B1_m", bufs=1)   # [x; -sq]
            nc.vector.memset(A1_m[:], 1.0)    # row 3 keeps +1
            for j in range(6):
                pt = ps(3, 512)
                for q in range(4):
                    b = 4 * j + q
                    nc.tensor.matmul(pt[:, 128 * q:128 * (q + 1)],
                                     xyz_sb[:, b:b + 1, :], ident[:, :],
                                     is_transpose=True)
                nc.scalar.mul(A1_m[0:3, 512 * j:512 * (j + 1)], pt[:], 2.0)
                nc.vector.tensor_copy(B1_m[0:3, 512 * j:512 * (j + 1)], pt[:])
                nc.scalar.copy(fxT[C:C + 3, 512 * j:512 * (j + 1)], pt[:])

            xyz2 = small.tile([128, NB, 3], f32)
            nc.vector.tensor_tensor(xyz2[:], xyz_sb[:], xyz_sb[:], Alu.mult)
            sq_p = small.tile([128, NB, 1], f32)
            nc.vector.tensor_reduce(sq_p[:], xyz2[:], axis=AX.X, op=Alu.add)
            sqn_p = small.tile([128, NB, 1], f32)
            nc.vector.tensor_scalar(sqn_p[:], sq_p[:], -1.0, None, Alu.mult)
            pt = ps(NB, 128)
            nc.tensor.matmul(pt[:], sq_p[:], ident[:, :], is_transpose=True)
            sq24 = small.tile([NB, 128], f32)
            nc.vector.tensor_copy(sq24[:], pt[:])
            pt = ps(NB, 128)
            nc.tensor.matmul(pt[:], sqn_p[:], ident[:, :], is_transpose=True)
            sqn24 = small.tile([NB, 128], f32)
            nc.vector.tensor_copy(sqn24[:], pt[:])
            nc.sync.dma_start(B1_m[3:4, :], sqn24[:])

            headsT = big.tile([5, N], f32)
            gateT = big.tile([C, N], f32)
            for j in range(6):
                sl = slice(512 * j, 512 * (j + 1))
                ph = ps(5, 512)
                nc.tensor.matmul(ph[:], WhT[:], fT[:, sl], start=True, stop=True)
                nc.vector.tensor_scalar(headsT[:, sl], ph[:], bhead[:], None, Alu.add)
                pg = ps(C, 512)
                nc.tensor.matmul(pg[:], WgateT[:], fT[:, sl], start=True, stop=True)
                nc.scalar.activation(gateT[:, sl], pg[:], Act.Sigmoid,
                                     bias=bgate_v[:], scale=1.0)

            hp = small.tile([128, NB, 5], f32)
            pt5 = ps(128, NB * 5)
            for b in range(NB):
                nc.tensor.matmul(pt5[:, 5 * b:5 * (b + 1)],
                                 headsT[:, 128 * b:128 * (b + 1)], ident[0:5, 0:5],
                                 is_transpose=True)
            nc.vector.tensor_copy(hp[:], pt5[:])

            flow_p = hp[:, :, 0:3]
            de = small.tile([128, NB, 1], f32)
            tmp_b = small.tile([128, NB, 1], f32)
            nc.scalar.activation(tmp_b[:], hp[:, :, 3:4], Act.Exp,
                                 bias=zero128[:], scale=1.0)
            nc.vector.tensor_scalar(tmp_b[:], tmp_b[:], 1.0, None, Alu.add)
            nc.scalar.activation(tmp_b[:], tmp_b[:], Act.Ln,
                                 bias=zero128[:], scale=1.0)
            sgu = small.tile([128, NB, 1], f32)
            nc.scalar.activation(sgu[:], hp[:, :, 4:5], Act.Sigmoid,
                                 bias=zero128[:], scale=1.0)
            nc.vector.tensor_scalar(sgu[:], sgu[:], 1.0, None, Alu.add)
            nc.vector.tensor_tensor(de[:], tmp_b[:], sgu[:], Alu.mult)
            de16 = small.tile([128, NB, 1], f32)
            nc.vector.tensor_scalar(de16[:], de[:], 1.0 / K, None, Alu.mult)

            fl2 = small.tile([128, NB, 3], f32)
            nc.vector.tensor_tensor(fl2[:], flow_p, flow_p, Alu.mult)
            vn = small.tile([128, NB, 1], f32)
            nc.vector.tensor_reduce(vn[:], fl2[:], axis=AX.X, op=Alu.add)
            nc.scalar.activation(vn[:], vn[:], Act.Sqrt, bias=zero128[:], scale=1.0)
            nc.vector.tensor_scalar(vn[:], vn[:], 1e-6, None, Alu.max)
            rv = small.tile([128, NB, 1], f32)
            nc.vector.reciprocal(rv[:], vn[:])
            vhat = small.tile([128, NB, 3], f32)
            nc.vector.tensor_tensor(vhat[:], flow_p,
                                    rv[:].broadcast_to((128, NB, 3)), Alu.mult)

            fgm = small.tile([3, 1], f32)
            nc.vector.tensor_reduce(fgm[:], headsT[0:3, :], axis=AX.X, op=Alu.add)
            nc.vector.tensor_scalar(fgm[:], fgm[:], 1.0 / N, None, Alu.mult)
            pg1 = ps(C, 1)
            nc.tensor.matmul(pg1[:], Wg1T[:], fgm[:], start=True, stop=True)
            hg = small.tile([C, 1], f32)
            nc.scalar.activation(hg[:], pg1[:], Act.Relu, bias=bg1_v[:], scale=1.0)
            pg2 = ps(C, 1)
            nc.tensor.matmul(pg2[:], Wg2T[:], hg[:], start=True, stop=True)
            fgf = small.tile([C, 1], f32)
            nc.vector.tensor_scalar(fgf[:], pg2[:], bg2_v[:], None, Alu.add)
            # TR = gate * fgf, transposed to point layout early (overlaps pass A)
            nc.vector.tensor_scalar(gateT[:], gateT[:], fgf[:], None, Alu.mult)
            TRp = big.tile([128, NB, C], f32)
            for j in range(3):
                pt = ps(128, 512)
                for q in range(8):
                    b = 8 * j + q
                    nc.tensor.matmul(pt[:, C * q:C * (q + 1)],
                                     gateT[:, 128 * b:128 * (b + 1)],
                                     ident[0:C, 0:C], is_transpose=True)
                nc.scalar.copy(TRp[:, 8 * j:8 * (j + 1), :], pt[:])

            # ---------------- S2 pass A -----------------
            idx_all = big.tile([128, NB * K], u16)
            for b in range(NB):
                negd = pa.tile([128, N], f32, tag="negd")
                for j in range(6):
                    pd = ps(128, 512)
                    nc.tensor.matmul(pd[:], A1_m[:, 128 * b:128 * (b + 1)],
                                     B1_m[:, 512 * j:512 * (j + 1)],
                                     start=True, stop=True)
                    nc.scalar.activation(negd[:, 512 * j:512 * (j + 1)], pd[:],
                                         Act.Identity,
                                         bias=sqn_p[:, b:b + 1, 0:1].rearrange(
                                             "p a b -> p (a b)"),
                                         scale=1.0)
                if debug and b == 0:
                    nc.sync.dma_start(AP(dbg_ext["d_negd"], 0, [[N, 128], [1, N]]),
                                      negd[:])
                cand = small.tile([128, 64], f32, tag="cand", bufs=2)
                segloc = small.tile([128, 64], u16, tag="segloc", bufs=2)
                for s8 in range(8):
                    nc.vector.max(cand[:, 8 * s8:8 * (s8 + 1)],
                                  negd[:, 384 * s8:384 * (s8 + 1)])
                    nc.vector.max_index(segloc[:, 8 * s8:8 * (s8 + 1)],
                                        cand[:, 8 * s8:8 * (s8 + 1)],
                                        negd[:, 384 * s8:384 * (s8 + 1)])
                jc16 = small.tile([128, 64], u16, tag="jc16", bufs=2)
                nc.vector.tensor_tensor(jc16[:], segloc[:], segb64u[:], Alu.add)
                v16 = small.tile([128, 16], f32, tag="v16", bufs=2)
                mrc = small.tile([128, 64], f32, tag="mrc", bufs=2)
                cp16 = small.tile([128, 16], u16, tag="cp16", bufs=2)
                nc.vector.max(v16[:, 0:8], cand[:])
                nc.vector.max_index(cp16[:, 0:8], v16[:, 0:8], cand[:])
                nc.vector.match_replace(mrc[:], v16[:, 0:8], cand[:], -1e30)
                nc.vector.max(v16[:, 8:16], mrc[:])
                nc.vector.max_index(cp16[:, 8:16], v16[:, 8:16], mrc[:])
                rankmap = small.tile([128, 64], i16, tag="rankmap", bufs=2)
                nc.gpsimd.local_scatter(rankmap[:], rank16[:],
                                        cp16[:].bitcast(i16),
                                        channels=128, num_elems=64, num_idxs=16)
                nc.vector.tensor_scalar(rankmap[:], rankmap[:], 1, None,
                                        Alu.subtract)
                nc.gpsimd.local_scatter(idx_all[:, K * b:K * (b + 1)].bitcast(i16),
                                        jc16[:].bitcast(i16), rankmap[:],
                                        channels=128, num_elems=16, num_idxs=64)

            # ---------------- S3 gather prep -----------------
            # Stage idx to DRAM so that each gather call (bgrp, k) reads a
            # contiguous wrapped [16, 32] block:
            #   dram2 addr = ((b//BT)*K + k)*512 + (p%16)*32 + (b%BT)*8 + p//16
            NBG = NB // BT
            idx_dram = dram.tile([NBG * K * 512], i16)
            for ph in range(8):
                for bg in range(NBG):
                    nc.sync.dma_start(
                        AP(idx_dram.tensor, bg * 512 * K + ph,
                           [[32, 16], [8, BT], [512, K]]),
                        idx_all[16 * ph:16 * (ph + 1),
                                bg * BT * K:(bg + 1) * BT * K].bitcast(i16)
                        .rearrange("p (bl k) -> p bl k", k=K))
            idx_wrap = small.tile([128, NBG * K, 32], i16)
            for g in range(8):
                for bg in range(NBG):
                    nc.sync.dma_start(
                        idx_wrap[16 * g:16 * (g + 1), bg * K:(bg + 1) * K, :],
                        AP(idx_dram.tensor, bg * 512 * K,
                           [[32, 16], [512, K], [1, 32]]))

            if debug:
                idxf = small.tile([128, NB * K], f32, tag="idxf")
                nc.vector.tensor_copy(idxf[:], idx_all[:])
                nc.sync.dma_start(AP(dbg_ext["d_idx"], 0, [[NB * K, 128], [1, NB * K]]),
                                  idxf[:])

            # ---------------- S4 pass C -----------------
            agg = big.tile([128, NB, C], f32)
            dp = small.tile([128, NB, 2], f32)
            if debug:
                dnum = big.tile([128, NB, K], f32, tag="dnum")
                duw = big.tile([128, NB, K], f32, tag="duw")
            pc = ctx.enter_pc(tc.tile_pool(name="passc", bufs=2))
            pcw = ctx.enter_pc(tc.tile_pool(name="passcw", bufs=1))
            for s in range(NSL):
                b0 = BT * s
                fnei = pc.tile([128, BT, K, 80], f32, tag="fnei", bufs=1)
                gth = pc.tile([128, K * BT * 128], f32, tag="gth", bufs=1)
                nc.gpsimd.ap_gather(
                    gth[:],
                    fxT[:],
                    idx_wrap[:, s * K:(s + 1) * K, :].rearrange("p a q -> p (a q)"),
                    channels=128, num_elems=N, d=1, num_idxs=K * BT * 128)
                for kq in range(K):
                    ptg = ps(128, BT * 128)
                    for q in range(BT):
                        nc.tensor.matmul(
                            ptg[:, 128 * q:128 * (q + 1)],
                            gth[:, kq * BT * 128 + 128 * q:
                                kq * BT * 128 + 128 * (q + 1)],
                            ident[:, :], is_transpose=True)
                    nc.scalar.copy(fnei[:, :, kq:kq + 1, :],
                                   ptg[:].rearrange("p (b c) -> p b c", c=128)[:, :, 0:80])
                xyz_nei = fnei[:, :, :, C:C + 3]
                f_nei = fnei[:, :, :, 0:C]
                xsl = xyz_sb[:, b0:b0 + BT, :]
                dxyz = pcw.tile([128, BT, K, 3], f32, tag="dxyz", bufs=2)
                nc.vector.tensor_tensor(
                    dxyz[:], xyz_nei,
                    xsl.unsqueeze(2).broadcast_to((128, BT, K, 3)), Alu.subtract)
                t3 = pcw.tile([128, BT, K, 3], f32, tag="t3", bufs=2)
                nc.vector.tensor_tensor(t3[:], dxyz[:], dxyz[:], Alu.mult)
                d2k = pcw.tile([128, BT, K], f32, tag="d2k", bufs=2)
                nc.vector.tensor_reduce(d2k[:], t3[:], axis=AX.X, op=Alu.add)
                sqd = pcw.tile([128, BT, K], f32, tag="sqd", bufs=2)
                nc.scalar.activation(sqd[:], d2k[:], Act.Ln,
                                     bias=zero128[:], scale=1.0)
                nc.scalar.activation(sqd[:], sqd[:], Act.Exp,
                                     bias=zero128[:], scale=0.5)
                den = pcw.tile([128, BT, K], f32, tag="den", bufs=2)
                nc.vector.tensor_scalar(den[:], sqd[:], 1e-6, None, Alu.max)
                rden = pcw.tile([128, BT, K], f32, tag="rden", bufs=2)
                nc.vector.reciprocal(rden[:], den[:])
                nc.vector.tensor_tensor(
                    t3[:], dxyz[:],
                    vhat[:, b0:b0 + BT, :].unsqueeze(2).broadcast_to((128, BT, K, 3)),
                    Alu.mult)
                numv = pcw.tile([128, BT, K], f32, tag="numv", bufs=2)
                nc.vector.tensor_reduce(numv[:], t3[:], axis=AX.X, op=Alu.add)
                if debug:
                    nc.vector.tensor_copy(dnum[:, b0:b0 + BT, :], numv[:])
                ek = pcw.tile([128, BT, K], f32, tag="ek", bufs=2)
                nc.vector.tensor_tensor(ek[:], numv[:], rden[:], Alu.mult)
                nc.scalar.activation(ek[:], ek[:], Act.Exp,
                                     bias=zero128[:], scale=1.0 / TAU)
                se = pcw.tile([128, BT, 1], f32, tag="se", bufs=2)
                nc.vector.tensor_reduce(se[:], ek[:], axis=AX.X, op=Alu.add)
                rse = pcw.tile([128, BT, 1], f32, tag="rse", bufs=2)
                nc.vector.reciprocal(rse[:], se[:])
                uw = pcw.tile([128, BT, K], f32, tag="uw", bufs=2)
                nc.vector.tensor_tensor(uw[:], ek[:],
                                        rse[:].broadcast_to((128, BT, K)), Alu.mult)
                nc.vector.tensor_tensor(
                    uw[:], uw[:],
                    de16[:, b0:b0 + BT, :].broadcast_to((128, BT, K)), Alu.add)
                if debug:
                    nc.vector.tensor_copy(duw[:, b0:b0 + BT, :], uw[:])
                    if s == 0:
                        nc.sync.dma_start(
                            AP(dbg_ext["d_fnei"], 0,
                               [[BT * K * 80, 128], [1, BT * K * 80]]), fnei[:])
                prod = pcw.tile([128, BT, K, C], f32, tag="prod")
                nc.vector.tensor_tensor(
                    prod[:], f_nei,
                    uw[:].unsqueeze(3).broadcast_to((128, BT, K, C)), Alu.mult)
                s1 = pcw.tile([128, BT, 8, C], f32, tag="s1")
                prodv = prod[:].rearrange("p b (k2 two) c -> p b k2 (two c)", two=2)
                nc.vector.tensor_tensor(s1[:], prodv[:, :, :, 0:C],
                                        prodv[:, :, :, C:2 * C], Alu.add)
                s2 = pcw.tile([128, BT, 4, C], f32, tag="s2")
                s1v = s1[:].rearrange("p b (k2 two) c -> p b k2 (two c)", two=2)
                nc.vector.tensor_tensor(s2[:], s1v[:, :, :, 0:C],
                                        s1v[:, :, :, C:2 * C], Alu.add)
                s3 = pcw.tile([128, BT, 2, C], f32, tag="s3")
                s2v = s2[:].rearrange("p b (k2 two) c -> p b k2 (two c)", two=2)
                nc.vector.tensor_tensor(s3[:], s2v[:, :, :, 0:C],
                                        s2v[:, :, :, C:2 * C], Alu.add)
                s3v = s3[:].rearrange("p b (one two) c -> p b one (two c)", two=2)
                nc.vector.tensor_tensor(agg[:, b0:b0 + BT, :], s3v[:, :, :, 0:C],
                                        s3v[:, :, :, C:2 * C], Alu.add)
                # dist stats
                ndsl = dp[:, b0:b0 + BT, 0:1]
                nvsl = dp[:, b0:b0 + BT, 1:2]
                nc.vector.tensor_reduce(ndsl, sqd[:], axis=AX.X, op=Alu.add)
                nc.vector.tensor_scalar(ndsl, ndsl, 1.0 / K, None, Alu.mult)
                d2m = pcw.tile([128, BT, 1], f32, tag="d2m", bufs=2)
                nc.vector.tensor_reduce(d2m[:], d2k[:], axis=AX.X, op=Alu.add)
                nc.vector.tensor_scalar(d2m[:], d2m[:], 1.0 / K, None, Alu.mult)
                nd2 = pcw.tile([128, BT, 1], f32, tag="nd2", bufs=2)
                nc.vector.tensor_tensor(nd2[:], ndsl, ndsl, Alu.mult)
                nc.vector.tensor_tensor(nvsl, d2m[:], nd2[:], Alu.subtract)

            if debug:
                nc.sync.dma_start(AP(dbg_ext["d_agg"], 0, [[NB * C, 128], [1, NB * C]]),
                                  agg[:])
                nc.sync.dma_start(AP(dbg_ext["d_de"], 0, [[NB, 128], [1, NB]]), de[:])
                nc.sync.dma_start(AP(dbg_ext["d_dist"], 0, [[NB * 2, 128], [1, NB * 2]]),
                                  dp[:])
                nc.sync.dma_start(AP(dbg_ext["d_heads"], 0, [[N, 5], [1, N]]),
                                  headsT[:])
                nc.sync.dma_start(AP(dbg_ext["d_num"], 0, [[NB * K, 128], [1, NB * K]]), dnum[:])
                nc.sync.dma_start(AP(dbg_ext["d_uw"], 0, [[NB * K, 128], [1, NB * K]]), duw[:])
                nc.sync.dma_start(AP(dbg_ext["d_vhat"], 0, [[NB * 3, 128], [1, NB * 3]]), vhat[:])

            # ---------------- S5 reaction + BN -----------------
            ctx.close_pc()
            late = ctx.enter_pa(tc.tile_pool(name="late", bufs=1))
            distT = late.tile([2, N], f32)
            for j in range(6):
                ptd = ps(2, 512)
                for q in range(4):
                    b = 4 * j + q
                    nc.tensor.matmul(ptd[:, 128 * q:128 * (q + 1)],
                                     dp[:, b:b + 1, :], ident[:, :],
                                     is_transpose=True)
                nc.vector.tensor_copy(distT[:, 512 * j:512 * (j + 1)], ptd[:])

            x_sb = late.tile([C, N], f32)
            xs6 = small.tile([C, 6], f32)
            x2s6 = small.tile([C, 6], f32)
            scr = late.tile([C, N], f32)
            for j in range(6):
                sl = slice(512 * j, 512 * (j + 1))
                px = ps(C, 512)
                nc.tensor.matmul(px[:], Wr1fT[:], fT[:, sl], start=True, stop=False)
                nc.tensor.matmul(px[:], Wr1flT[:], headsT[0:3, sl],
                                 start=False, stop=False)
                nc.tensor.matmul(px[:], Wr1dT[:], distT[:, sl],
                                 start=False, stop=True)
                nc.scalar.activation(x_sb[:, sl], px[:], Act.Copy, bias=0.0,
                                     scale=1.0, accum_out=xs6[:, j:j + 1])
                nc.scalar.activation(scr[:, sl], x_sb[:, sl], Act.Square,
                                     bias=zero128[0:C, :], scale=1.0,
                                     accum_out=x2s6[:, j:j + 1])
            bn_loc = small.tile([C, 2], f32)
            nc.vector.tensor_reduce(bn_loc[:, 0:1], xs6[:], axis=AX.X, op=Alu.add)
            nc.vector.tensor_reduce(bn_loc[:, 1:2], x2s6[:], axis=AX.X, op=Alu.add)
            bn_in = dram.tile([C, 2], f32)
            bn_out = dram.tile([C, 2], f32)
            bn_g = small.tile([C, 2], f32)
            if nocol:
                nc.vector.tensor_scalar(bn_g[:], bn_loc[:], float(NCORES), None,
                                        Alu.mult)
            else:
                nc.sync.dma_start(bn_in[:], bn_loc[:])
                nc.gpsimd.collective_compute(
                    "AllReduce", Alu.add, replica_groups=[list(range(NCORES))],
                    ins=[bn_in[:].opt()], outs=[bn_out[:].opt()])
                nc.sync.dma_start(bn_g[:], bn_out[:])
            Mtot = float(NCORES * N)
            mu = small.tile([C, 1], f32)
            nc.vector.tensor_scalar(mu[:], bn_g[:, 0:1], 1.0 / Mtot, None, Alu.mult)
            var = small.tile([C, 1], f32)
            nc.vector.tensor_scalar(var[:], bn_g[:, 1:2], 1.0 / Mtot, None, Alu.mult)
            mu2 = small.tile([C, 1], f32)
            nc.vector.tensor_tensor(mu2[:], mu[:], mu[:], Alu.mult)
            nc.vector.tensor_tensor(var[:], var[:], mu2[:], Alu.subtract)
            nc.vector.tensor_scalar(var[:], var[:], BN_EPS, None, Alu.add)
            nc.scalar.activation(var[:], var[:], Act.Sqrt,
                                 bias=zero128[0:C, :], scale=1.0)
            rstd = small.tile([C, 1], f32)
            nc.vector.reciprocal(rstd[:], var[:])
            s_vec = small.tile([C, 1], f32)
            nc.vector.tensor_tensor(s_vec[:], gamma_v[:], rstd[:], Alu.mult)
            b_vec = small.tile([C, 1], f32)
            nc.vector.tensor_tensor(b_vec[:], mu[:], s_vec[:], Alu.mult)
            nc.vector.tensor_tensor(b_vec[:], beta_v[:], b_vec[:], Alu.subtract)
            if debug:
                nc.sync.dma_start(AP(dbg_ext["d_bn"], 0, [[4, C], [1, 2]]), bn_g[:])
                nc.sync.dma_start(AP(dbg_ext["d_bn"], 2, [[4, C], [1, 1]]), mu[:])
                nc.sync.dma_start(AP(dbg_ext["d_bn"], 3, [[4, C], [1, 1]]), var[:])

            nc.scalar.activation(x_sb[:], x_sb[:], Act.Relu,
                                 bias=b_vec[:], scale=s_vec[:])
            for j in range(6):
                sl = slice(512 * j, 512 * (j + 1))
                pr = ps(C, 512)
                nc.tensor.matmul(pr[:], Wr2T[:], x_sb[:, sl], start=True, stop=True)
                nc.scalar.copy(scr[:, sl], pr[:])
            nc.vector.tensor_scalar(scr[:], scr[:], br2_v[:], None, Alu.add)

            # ---------------- S6 final combine -----------------
            de1 = small.tile([128, NB, 1], f32)
            nc.vector.tensor_scalar(de1[:], de[:], 1.0, None, Alu.add)
            out_sb = late.tile([128, NB, C], f32)
            nc.vector.tensor_tensor(out_sb[:], f_sb[:],
                                    de1[:].broadcast_to((128, NB, C)), Alu.mult)
            nc.vector.tensor_tensor(agg[:], agg[:], out_sb[:], Alu.subtract)
            nc.vector.tensor_tensor(agg[:], agg[:], TRp[:], Alu.add)
            for j in range(3):
                pt = ps(128, 512)
                for q in range(8):
                    b = 8 * j + q
                    nc.tensor.matmul(pt[:, C * q:C * (q + 1)],
                                     scr[:, 128 * b:128 * (b + 1)],
                                     ident[0:C, 0:C], is_transpose=True)
                nc.scalar.copy(out_sb[:, 8 * j:8 * (j + 1), :], pt[:])
            nc.vector.tensor_tensor(agg[:], agg[:], out_sb[:], Alu.add)
            nc.vector.scalar_tensor_tensor(out_sb[:], agg[:], dtv[:], f_sb[:],
                                           Alu.mult, Alu.add)
            nc.sync.dma_start(AP(out_ext, 0, [[C, 128], [128 * C, NB], [1, C]]),
                              out_sb[:])

    nc.compile()
    return nc


@functools.cache
def _get_nc(debug=False):
    return _build(debug=debug)


def _run(nc, inputs, trace=False):
    from concourse.bass_utils import run_bass_kernel_spmd
    f_seq = np.ascontiguousarray(np.asarray(inputs["f_seq"], dtype=np.float32))
    xyz = np.ascontiguousarray(np.asarray(inputs["xyz"], dtype=np.float32))
    in_maps = []
    for core in range(NCORES):
        b, l = divmod(core, L)
        m = {"f": f_seq[b, l], "xyz": xyz[b, l]}
        for k in WEIGHT_NAMES:
            m[k] = np.ascontiguousarray(
                np.asarray(inputs[k], dtype=np.float32).reshape(-1))
        in_maps.append(m)
    return run_bass_kernel_spmd(nc, in_maps, core_ids=list(range(NCORES)),
                                trace=trace)


def kernel(**inputs):
    nc = _get_nc()
    res = _run(nc, inputs)
    out = np.stack([np.asarray(res.results[i]["out"]) for i in range(NCORES)])
    return out.reshape(B, L, N, C).astype(np.float32)
